# revision 1
# baseline (speedup 1.0000x reference)
"""CrossAttn + TISA bias kernel for TRN2, 8-core SPMD.

Sharding: core = (batch b = core//2, query half = core%2).
Each core computes the full kv projection for its batch (duplicated within
the pair) and its 512 query rows end-to-end. No collectives.

v2 vs v1:
  - softmax denominator fused into the attn matmul: stationary [v_h | 1]
    (M=65) so each wT tile streams through the PE once, not twice
  - per-head reciprocal broadcast via a K=1 ones matmul (bf16)
  - odd heads' normalized attn moved to partitions 64..127 with a small
    SBUF->SBUF DMA (engines cannot shift partitions)
  - weight DMAs reordered/column-chunked so the first q-proj matmul starts
    after ~0.4 MB instead of 9 MB
  - gate phase emits 512-col output halves, sigmoid path first, so the
    tail after the last matmul is short

Inputs arrive host-transposed: xqt/xkvt are [d_in, tokens].
  qT:   [d_out(part), i]   (scaled by 1/sqrt(Dh))
  kT:   [d_out(part), j]
  vaug: [j(part), jc, h, 65] = v columns 0..63, ones column 64
  S^T:  [j(part), i] = kT_h.T @ qT_h          (K=64)
  wT = exp(S^T) * srow[:, C:C+512]            (shifted exp-bias table slice)
  ps_h = [vaug_h]^T @ wT accumulated over jc  -> rows 0..63 attn, row 64 sums
  rb = ones1^T @ (1/sums)                     (K=1 PE broadcast)
  attn = ps_h * rb                            (per-head normalize)
  gate: attn.T @ Wg -> [i(part), 2048]; out = (a+bga)*sigmoid(b+bgb)
"""

import numpy as np
import ml_dtypes

import concourse.bacc as bacc
import concourse.mybir as mybir
import concourse.tile as tile
from concourse.bass import ts

L = 1024
D = 1024
H = 16
DH = 64
LQ = 512          # q rows per core
NIC = LQ // 128   # 4 i-chunks
NJC = L // 128    # 8 j-chunks
NKC = D // 128    # 8 d_model chunks
SROW_W = 1408
NUM_KERNELS = 21

F32 = mybir.dt.float32
BF16 = mybir.dt.bfloat16
EXP = mybir.ActivationFunctionType.Exp
SIG = mybir.ActivationFunctionType.Sigmoid
CPY = mybir.ActivationFunctionType.Copy
MUL = mybir.AluOpType.mult
ADD = mybir.AluOpType.add

_NP = {"f32": np.float32, "bf16": ml_dtypes.bfloat16}


def ds2(hh):
    return slice(hh * 64, hh * 64 + 64)


def build_nc(cfg="bf16"):
    mdt = BF16

    nc = bacc.Bacc("TRN2", target_bir_lowering=False, debug=False, num_devices=8)

    xqt_d = nc.dram_tensor("xqt", [D, LQ], mdt, kind="ExternalInput").ap()
    xkvt_d = nc.dram_tensor("xkvt", [D, L], mdt, kind="ExternalInput").ap()
    wqc_d = nc.dram_tensor("wqc", [NKC, D, 128], mdt, kind="ExternalInput").ap()
    wmk_d = nc.dram_tensor("wmk", [NKC, D, 128], mdt, kind="ExternalInput").ap()
    wmv_d = nc.dram_tensor("wmv", [D, D], mdt, kind="ExternalInput").ap()
    wg_d = nc.dram_tensor("wg", [D, 2 * D], mdt, kind="ExternalInput").ap()
    srow_d = nc.dram_tensor("srow", [H, 128, SROW_W], mdt, kind="ExternalInput").ap()
    bg_d = nc.dram_tensor("bgrep", [128, 2 * D], F32, kind="ExternalInput").ap()
    out_d = nc.dram_tensor("out", [LQ, D], F32, kind="ExternalOutput").ap()

    with tile.TileContext(nc) as tc:
        with (
            tc.tile_pool(name="const", bufs=1) as constp,
            tc.tile_pool(name="persist", bufs=1) as pers,
            tc.tile_pool(name="psum", bufs=1, space="PSUM") as psum,
        ):
            onesc = constp.tile([128, 64], mdt)
            nc.gpsimd.memset(onesc, 1.0)

            # PE warmup: keep the array busy during the initial input DMA so
            # the first real matmuls run at full clock (pstate ramp).
            dummy = constp.tile([128, 512], mdt)
            nc.gpsimd.memset(dummy, 0.0)
            for _ in range(10):
                ps_w = psum.tile([128, 512], F32, tag="t1", bufs=2)
                nc.tensor.matmul(ps_w, dummy[:, 0:128], dummy, start=True, stop=True)

            qT = pers.tile([128, NKC, LQ], mdt)        # [d_out, mc, i]
            kT = pers.tile([128, NKC, L], mdt)         # [d_out, mc, j]
            vaug = pers.tile([128, NJC, H, 65], mdt)   # [j, jc, h, v|1]
            attn = pers.tile([128, NKC, LQ], mdt)      # [d_model, chunk, i]

            srow_tiles = {}

            def issue_srow(h):
                srow_tiles[h] = pers.tile(
                    [128, SROW_W], mdt, tag="srow", bufs=4, name=f"srow{h}")
                nc.sync.dma_start(out=srow_tiles[h], in_=srow_d[h, :, :])

            # scores + exp + bias-mul for one (head, jc-pair); wT tiles are
            # stashed in `pending` so the attn matmul can run later
            pending = {}

            def scores_unit(c, hh, jp, srow_sb):
                h = 2 * c + hh
                ps_s = psum.tile([128, 2 * LQ], F32, tag="ss", bufs=2,
                                 name="ps_s")
                for t in range(2):
                    jc = 2 * jp + t
                    nc.tensor.matmul(
                        ps_s[:, ts(t, LQ)],
                        kT[ds2(hh), c, ts(jc, 128)], qT[ds2(hh), c, :],
                        start=True, stop=True)
                wexp = pers.tile([128, 2 * LQ], mdt, tag="wexp", bufs=3,
                                 name="wexp")
                nc.scalar.activation(wexp, ps_s, EXP)
                for t in range(2):
                    jc = 2 * jp + t
                    wT = pers.tile([128, LQ], mdt, tag="wt", bufs=34,
                                   name="wT")
                    C0 = 896 - jc * 128
                    nc.vector.tensor_tensor(
                        wT, wexp[:, ts(t, LQ)], srow_sb[:, C0:C0 + LQ], MUL)
                    pending[(h, jc)] = wT

            # =========== phase B: projections ==========
            with tc.tile_pool(name="phB", bufs=1) as phb:
                wqc = phb.tile([128, NKC, NKC, 128], mdt)   # [k, mc, kc, col]
                xqT = phb.tile([128, NKC, LQ], mdt)         # [d_in, kc, i]
                xkvT = phb.tile([128, NKC, L], mdt)         # [d_in, kc, j]
                wmk = phb.tile([128, NKC, NKC, 128], mdt)
                wmv = phb.tile([128, NKC, D], mdt)          # [k, kc, v-cols]

                # Weights stream on the Pool SWDGE queue; xqT on SP, xkvT on
                # the Act HWDGE queue. Three queues issue in parallel so the
                # first q-proj matmul starts after ~0.4 MB, not 9 MB.
                for mc in range(NKC):
                    nc.gpsimd.dma_start(
                        out=wqc[:, mc],
                        in_=wqc_d[mc].rearrange("(kc p) c -> p kc c", p=128))
                for mc in range(NKC):
                    nc.gpsimd.dma_start(
                        out=wmk[:, mc],
                        in_=wmk_d[mc].rearrange("(kc p) c -> p kc c", p=128))
                for kc in range(NKC):
                    nc.gpsimd.dma_start(out=wmv[:, kc, :], in_=wmv_d[ts(kc, 128), :])
                for kc in range(NKC):
                    nc.sync.dma_start(out=xqT[:, kc, :], in_=xqt_d[ts(kc, 128), :])
                for kc in range(NKC):
                    nc.scalar.dma_start(out=xkvT[:, kc, :], in_=xkvt_d[ts(kc, 128), :])

                for jc in range(NJC):
                    nc.gpsimd.memset(vaug[:, jc, :, 64], 1.0)

                # q projection
                for mc in range(NKC):
                    ps = psum.tile([128, LQ], F32, tag="t1", bufs=2)
                    for kc in range(NKC):
                        nc.tensor.matmul(
                            ps, wqc[:, mc, kc, :], xqT[:, kc, :],
                            start=(kc == 0), stop=(kc == NKC - 1))
                    nc.scalar.activation(qT[:, mc, :], ps, CPY, scale=0.125)

                # k projection
                for mc in range(NKC):
                    for nh in range(2):
                        ps = psum.tile([128, 512], F32, tag="t1", bufs=2)
                        for kc in range(NKC):
                            nc.tensor.matmul(
                                ps, wmk[:, mc, kc, :], xkvT[:, kc, ts(nh, 512)],
                                start=(kc == 0), stop=(kc == NKC - 1))
                        if nh == 0:
                            nc.vector.tensor_copy(kT[:, mc, ts(nh, 512)], ps)
                        else:
                            nc.scalar.activation(kT[:, mc, ts(nh, 512)], ps, CPY)

                for h in range(4):
                    issue_srow(h)

                # v projection interleaved with pairs 0-1 scores/exp/bias-mul:
                # the Act-bound exp pipeline hides under v-proj PE work. The
                # attn matmuls for these pairs run after v-proj (deferred).
                vgroups = [(jc, nh) for jc in range(NJC) for nh in range(2)]
                vi = 0
                for c in range(2):
                    for hh in range(2):
                        h = 2 * c + hh
                        srow_sb = srow_tiles.pop(h)
                        for jp in range(NJC // 2):
                            scores_unit(c, hh, jp, srow_sb)
                            jcv, nh = vgroups[vi]
                            vi += 1
                            ps = psum.tile([128, 512], F32, tag="t1", bufs=2)
                            for kc in range(NKC):
                                nc.tensor.matmul(
                                    ps, xkvT[:, kc, ts(jcv, 128)],
                                    wmv[:, kc, ts(nh, 512)],
                                    start=(kc == 0), stop=(kc == NKC - 1))
                            dst = vaug[:, jcv, nh * 8:(nh + 1) * 8, 0:64]
                            if nh == 0:
                                nc.vector.tensor_copy(dst, ps)
                            else:
                                nc.scalar.activation(dst, ps, CPY)
                        issue_srow(h + 4)

            # ================= phase C: attention =================
            with tc.tile_pool(name="phC", bufs=1) as phc:
                # gate-phase weights load during C (Pool SWDGE queue)
                wg_r = phc.tile([128, NKC, 2 * D], mdt)
                bg_sb = phc.tile([128, 2 * D], F32)
                for kc in range(NKC):
                    nc.gpsimd.dma_start(out=wg_r[:, kc, :], in_=wg_d[ts(kc, 128), :])
                nc.gpsimd.dma_start(out=bg_sb, in_=bg_d)

                def finish_head(c, hh, ps_h):
                    rsb = phc.tile([128, LQ], mdt, tag="rsb", bufs=2,
                                   name="rsb")
                    with nc.allow_low_precision(reason="softmax recip bf16"):
                        nc.vector.reciprocal(rsb[64:65, :], ps_h[64:65, :])
                    rb_ps = psum.tile([64, LQ], F32, tag="t1", bufs=2,
                                      name="rb_ps")
                    nc.tensor.matmul(
                        rb_ps, onesc[64:65, :], rsb[64:65, :],
                        start=True, stop=True)
                    rb_sb = phc.tile([64, LQ], F32, tag="rbs", bufs=2,
                                     name="rb_sb")
                    nc.vector.tensor_copy(rb_sb, rb_ps)
                    if hh == 0:
                        nc.vector.tensor_tensor(
                            attn[0:64, c, :], ps_h[0:64, :], rb_sb, MUL)
                    else:
                        todd = phc.tile([64, LQ], mdt, tag="todd", bufs=2,
                                        name="todd")
                        nc.vector.tensor_tensor(todd, ps_h[0:64, :], rb_sb, MUL)
                        nc.sync.dma_start(out=attn[64:128, c, :], in_=todd)

                # deferred attn for the interleaved pairs
                for c in range(2):
                    for hh in range(2):
                        h = 2 * c + hh
                        ps_h = psum.tile([65, LQ], F32, tag="psh", bufs=2)
                        for jc in range(NJC):
                            nc.tensor.matmul(
                                ps_h, vaug[:, jc, h, :], pending.pop((h, jc)),
                                start=(jc == 0), stop=(jc == NJC - 1))
                        finish_head(c, hh, ps_h)

                # remaining pairs: scores/exp/mul and attn tightly pipelined
                for c in range(2, NKC):
                    for hh in range(2):
                        h = 2 * c + hh
                        srow_sb = srow_tiles.pop(h)
                        if h + 4 < H:
                            issue_srow(h + 4)
                        ps_h = psum.tile([65, LQ], F32, tag="psh", bufs=2)
                        for jp in range(NJC // 2):
                            scores_unit(c, hh, jp, srow_sb)
                            for t in range(2):
                                jc = 2 * jp + t
                                nc.tensor.matmul(
                                    ps_h, vaug[:, jc, h, :],
                                    pending.pop((h, jc)),
                                    start=(jc == 0), stop=(jc == NJC - 1))
                        finish_head(c, hh, ps_h)

                # ================= phase D: gate =================
                for ic in range(NIC):
                    for qa in range(2):
                        ps_b = psum.tile([128, 512], F32, tag="t1", bufs=2)
                        for kc in range(NKC):
                            nc.tensor.matmul(
                                ps_b, attn[:, kc, ts(ic, 128)],
                                wg_r[:, kc, slice(D + qa * 512, D + qa * 512 + 512)],
                                start=(kc == 0), stop=(kc == NKC - 1))
                        tb = phc.tile([128, 512], F32, tag="tb", bufs=2)
                        nc.vector.tensor_tensor(
                            tb, ps_b, bg_sb[:, D + qa * 512:D + qa * 512 + 512], ADD)
                        tsg = phc.tile([128, 512], F32, tag="tsg", bufs=2)
                        nc.scalar.activation(tsg, tb, SIG)

                        ps_a = psum.tile([128, 512], F32, tag="t1", bufs=2)
                        for kc in range(NKC):
                            nc.tensor.matmul(
                                ps_a, attn[:, kc, ts(ic, 128)],
                                wg_r[:, kc, ts(qa, 512)],
                                start=(kc == 0), stop=(kc == NKC - 1))
                        last = (ic == NIC - 1) and (qa == 1)
                        if not last:
                            ta = phc.tile([128, 512], F32, tag="ta", bufs=2)
                            nc.vector.tensor_tensor(
                                ta, ps_a, bg_sb[:, ts(qa, 512)], ADD)
                            outh = phc.tile([128, 512], F32, tag="outt", bufs=3)
                            nc.vector.tensor_tensor(outh, ta, tsg, MUL)
                            nc.sync.dma_start(
                                out=out_d[ts(ic, 128), ts(qa, 512)], in_=outh)
                        else:
                            # final chunk in 128-col pieces for a short tail
                            for qt in range(4):
                                lo = qa * 512 + qt * 128
                                sl = slice(qt * 128, qt * 128 + 128)
                                ta = phc.tile([128, 128], F32, tag="ta2", bufs=2)
                                nc.vector.tensor_tensor(
                                    ta, ps_a[:, sl], bg_sb[:, lo:lo + 128], ADD)
                                outh = phc.tile([128, 128], F32, tag="outt2", bufs=2)
                                nc.vector.tensor_tensor(outh, ta, tsg[:, sl], MUL)
                                nc.sync.dma_start(
                                    out=out_d[ts(ic, 128), lo:lo + 128], in_=outh)

    nc.compile()
    return nc


# ======================= host side =======================

def _tisa_ebias(amp, off, sharp):
    d = np.arange(-(L - 1), L, dtype=np.float32)
    s = np.sum(
        amp[:, :, None].astype(np.float32)
        * np.exp(-np.abs(sharp)[:, :, None].astype(np.float32)
                 * (d[None, None, :] - off[:, :, None].astype(np.float32)) ** 2),
        axis=1, dtype=np.float32).astype(np.float32)
    return np.exp(s).astype(np.float32)


def make_host_inputs(inputs, cfg="bf16"):
    npdt = _NP["bf16"]
    x_q = np.asarray(inputs["x_q"])
    x_kv = np.asarray(inputs["x_kv"])
    wq = np.asarray(inputs["Wq"]).astype(npdt)
    wm = np.asarray(inputs["Wm"]).astype(npdt)
    wg = np.asarray(inputs["Wg"]).astype(npdt)
    bg = np.asarray(inputs["bg"]).astype(np.float32)

    ebias = _tisa_ebias(np.asarray(inputs["tisa_amp"]),
                        np.asarray(inputs["tisa_off"]),
                        np.asarray(inputs["tisa_sharp"]))

    p_i = np.arange(128)[:, None]
    m_i = np.arange(SROW_W)[None, :]
    srows = []
    for i_off in (0, 512):
        idx = p_i - m_i + (1919 - i_off)
        srows.append(np.ascontiguousarray(ebias[:, idx]).astype(npdt))

    # column-chunked (mc-major) layouts for early compute start
    wqc = np.ascontiguousarray(
        wq.reshape(D, NKC, 128).transpose(1, 0, 2))          # [mc, k, col]
    wmk = np.ascontiguousarray(
        wm[:, :D].reshape(D, NKC, 128).transpose(1, 0, 2))   # [mc, k, col]
    wmv = np.ascontiguousarray(wm[:, D:])                    # [k, v-col]

    bgrep = np.ascontiguousarray(np.broadcast_to(bg, (128, 2 * D))).astype(np.float32)

    in_maps = []
    for core in range(8):
        b, half = core // 2, core % 2
        in_maps.append({
            "xqt": np.ascontiguousarray(
                x_q[b, half * LQ:(half + 1) * LQ].T).astype(npdt),
            "xkvt": np.ascontiguousarray(x_kv[b].T).astype(npdt),
            "wqc": wqc, "wmk": wmk, "wmv": wmv, "wg": wg,
            "srow": srows[half],
            "bgrep": bgrep,
        })
    return in_maps


def assemble_output(results):
    out = np.empty((4, L, D), dtype=np.float32)
    for core in range(8):
        b, half = core // 2, core % 2
        out[b, half * LQ:(half + 1) * LQ] = results[core]["out"]
    return out


# ======================= public entry point =======================

_NC_CACHE = {}


def _get_nc(cfg):
    if cfg not in _NC_CACHE:
        _NC_CACHE[cfg] = build_nc(cfg)
    return _NC_CACHE[cfg]


def kernel(**inputs):
    """Full (unsharded) inputs -> full (4, 1024, 1024) float32 output.

    Shards over 8 NeuronCores: core = (batch, query-half). Host precomputes
    the TISA exp-bias lookup table and pre-transposes activations; all dense
    compute (projections, attention, gate) runs on-device in bf16 matmuls
    with fp32 accumulation.
    """
    from concourse.bass_utils import run_bass_kernel_spmd

    cfg = "bf16"
    nc = _get_nc(cfg)
    in_maps = make_host_inputs(inputs, cfg)
    res = run_bass_kernel_spmd(nc, in_maps, core_ids=list(range(8)))
    return assemble_output(res.results)



# revision 31
# speedup vs baseline: 1.0570x; 1.0570x over previous
"""CrossAttn + TISA bias kernel for TRN2, 8-core SPMD.

Sharding: core = (batch b = core//2, query half = core%2).
Each core computes the full kv projection for its batch (duplicated within
the pair) and its 512 query rows end-to-end. No collectives.

v3: the four big GEMMs (q/k/v projections, gate) run as compensated fp8
DoubleRow matmuls. Each operand X is split hi/lo: xh = fp8(X*s),
xl = fp8(X*s - xh), and X@W is computed as xh@wh + xh@wl + xl@wh (the
lo*lo term is dropped; ~2^-8 relative error, bf16-level). DoubleRow packs
two 128-deep k-tiles per instruction at 0.5 PE cycles/row, so the 3-term
scheme costs 0.75x the bf16 GEMM at bf16-level precision.

  x scale 8 (max |x*8| ~ 42), W scale 1024 (max ~110), fp8e4m3 max 240.
  attn is rescaled by 32 through the v-projection descale so its hi/lo
  split (done on device from the bf16 normalize output) stays clear of
  the fp8 subnormal floor.

Scores and the attention matmul stay bf16: softmax weights cannot be
quantized to fp8 (3.6% rms error), and both matmuls are output-bound so
fp8 would not make them cheaper anyway.

Engine budget: PE ~140us (bottleneck), Act ~88 (exp-dominated), DVE ~90
(wT muls at the 2x bf16-SBUF rate), Pool ~50 (SWDGE triggers, lo-subs,
nh=0 copies). Weights are host-pre-arranged so every weight DMA is
contiguous (128 descriptors) - SWDGE trigger cost is ~1us fixed per DMA
on the Pool engine, so weight DMAs are few and big; x stays in 2-kc
chunks matching the DoubleRow pair granularity for early start.
"""

import numpy as np
import ml_dtypes

import concourse.bacc as bacc
import concourse.mybir as mybir
import concourse.tile as tile
from concourse.bass import ts

L = 1024
D = 1024
H = 16
DH = 64
LQ = 512          # q rows per core
NIC = LQ // 128   # 4 i-chunks
NJC = L // 128    # 8 j-chunks
NKC = D // 128    # 8 d_model chunks
NKP = NKC // 2    # 4 k-chunk pairs (DoubleRow)
SROW_W = 1408
NUM_KERNELS = 21

SX = 8.0          # x hi/lo fp8 scale
SW = 1024.0       # W hi/lo fp8 scale
SA = 32.0         # attn rescale for the gate's fp8 split
QSC = 0.125 / (SX * SW)   # q descale including 1/sqrt(DH)
KSC = 1.0 / (SX * SW)
VSC = SA / (SX * SW)
GSC = 1.0 / (SA * SW)     # gate psum descale

F32 = mybir.dt.float32
BF16 = mybir.dt.bfloat16
FP8 = mybir.dt.float8e4
DR = mybir.MatmulPerfMode.DoubleRow
EXP = mybir.ActivationFunctionType.Exp
SIG = mybir.ActivationFunctionType.Sigmoid
CPY = mybir.ActivationFunctionType.Copy
MUL = mybir.AluOpType.mult
ADD = mybir.AluOpType.add
SUB = mybir.AluOpType.subtract

NPF8 = ml_dtypes.float8_e4m3


def ds2(hh):
    return slice(hh * 64, hh * 64 + 64)


def build_nc(cfg="fp8"):
    mdt = BF16

    nc = bacc.Bacc("TRN2", target_bir_lowering=False, debug=False, num_devices=8)

    # x: [d_in, tokens] fp8 hi/lo
    xqh_d = nc.dram_tensor("xqh", [D, LQ], FP8, kind="ExternalInput").ap()
    xql_d = nc.dram_tensor("xql", [D, LQ], FP8, kind="ExternalInput").ap()
    xkh_d = nc.dram_tensor("xkh", [D, L], FP8, kind="ExternalInput").ap()
    xkl_d = nc.dram_tensor("xkl", [D, L], FP8, kind="ExternalInput").ap()
    # weights pre-arranged to SBUF layout (contiguous DMA):
    #   wq/wmk: [mc, p, kc*128]  (column-chunk-major, partition-contig)
    wqh_d = nc.dram_tensor("wqh", [NKC, 128, D], FP8, kind="ExternalInput").ap()
    wql_d = nc.dram_tensor("wql", [NKC, 128, D], FP8, kind="ExternalInput").ap()
    wmkh_d = nc.dram_tensor("wmkh", [NKC, 128, D], FP8, kind="ExternalInput").ap()
    wmkl_d = nc.dram_tensor("wmkl", [NKC, 128, D], FP8, kind="ExternalInput").ap()
    #   wmv/wg: [p, kc, cols]
    wmvh_d = nc.dram_tensor("wmvh", [128, NKC, D], FP8, kind="ExternalInput").ap()
    wmvl_d = nc.dram_tensor("wmvl", [128, NKC, D], FP8, kind="ExternalInput").ap()
    wgh_d = nc.dram_tensor("wgh", [128, NKC, 2 * D], FP8, kind="ExternalInput").ap()
    wgl_d = nc.dram_tensor("wgl", [128, NKC, 2 * D], FP8, kind="ExternalInput").ap()
    srow_d = nc.dram_tensor("srow", [H, 128, SROW_W], mdt, kind="ExternalInput").ap()
    bg_d = nc.dram_tensor("bgrep", [128, 2 * D], F32, kind="ExternalInput").ap()
    out_d = nc.dram_tensor("out", [LQ, D], mdt, kind="ExternalOutput").ap()

    with tile.TileContext(nc) as tc:
        with (
            tc.tile_pool(name="const", bufs=1) as constp,
            tc.tile_pool(name="persist", bufs=1) as pers,
            tc.tile_pool(name="psum", bufs=1, space="PSUM") as psum,
        ):
            onesc = constp.tile([128, 64], mdt)
            nc.gpsimd.memset(onesc, 1.0)

            # PE warmup: keep the array busy during the initial input DMA so
            # the first real matmuls run at full clock (pstate ramp).
            dummy = constp.tile([128, 512], mdt)
            nc.gpsimd.memset(dummy, 0.0)
            for _ in range(14):
                ps_w = psum.tile([128, 512], F32, tag="t1", bufs=2)
                nc.tensor.matmul(ps_w, dummy[:, 0:128], dummy, start=True, stop=True)

            qT = pers.tile([128, NKC, LQ], mdt)        # [d_out, mc, i]
            kT = pers.tile([128, NKC, L], mdt)         # [d_out, mc, j]
            vaug = pers.tile([128, NJC, H, 65], mdt)   # [j, jc, h, v|1]
            attn_h = pers.tile([128, NKC, LQ], FP8)    # fp8 hi of attn*SA
            attn_l = pers.tile([128, NKC, LQ], FP8)    # fp8 lo

            srow_tiles = {}

            def issue_srow(h):
                srow_tiles[h] = pers.tile(
                    [128, SROW_W], mdt, tag="srow", bufs=4, name=f"srow{h}")
                nc.sync.dma_start(out=srow_tiles[h], in_=srow_d[h, :, :])

            # scores + exp + bias-mul for one (head, jc-pair); wT tiles are
            # stashed in `pending` so the attn matmul can run later
            pending = {}

            def scores_unit(c, hh, jp, srow_sb):
                h = 2 * c + hh
                ps_s = psum.tile([128, 2 * LQ], F32, tag="ss", bufs=2,
                                 name="ps_s")
                for t in range(2):
                    jc = 2 * jp + t
                    nc.tensor.matmul(
                        ps_s[:, ts(t, LQ)],
                        kT[ds2(hh), c, ts(jc, 128)], qT[ds2(hh), c, :],
                        start=True, stop=True)
                wexp = pers.tile([128, 2 * LQ], mdt, tag="wexp", bufs=3,
                                 name="wexp")
                nc.scalar.activation(wexp, ps_s, EXP)
                for t in range(2):
                    jc = 2 * jp + t
                    wT = pers.tile([128, LQ], mdt, tag="wt", bufs=34,
                                   name="wT")
                    C0 = 896 - jc * 128
                    nc.vector.tensor_tensor(
                        wT, wexp[:, ts(t, LQ)], srow_sb[:, C0:C0 + LQ], MUL)
                    pending[(h, jc)] = wT

            # =========== phase B: projections ==========
            with tc.tile_pool(name="phB", bufs=1) as phb:
                wqch = phb.tile([128, NKC, NKC, 128], FP8)  # [k, mc, kc, col]
                wqcl = phb.tile([128, NKC, NKC, 128], FP8)
                xqTh = phb.tile([128, NKC, LQ], FP8)        # [d_in, kc, i]
                xqTl = phb.tile([128, NKC, LQ], FP8)
                xkvh = phb.tile([128, NKC, L], FP8)         # [d_in, kc, j]
                xkvl = phb.tile([128, NKC, L], FP8)
                wmkh = phb.tile([128, NKC, NKC, 128], FP8)
                wmkl = phb.tile([128, NKC, NKC, 128], FP8)
                wmvh = phb.tile([128, NKC, D], FP8)         # [k, kc, v-cols]
                wmvl = phb.tile([128, NKC, D], FP8)

                def wload(dst, src, mcs):
                    # weight chunk-group load: [n_mc, 128, D] -> [128, n_mc, D]
                    nc.gpsimd.dma_start(
                        out=dst[:, mcs[0]:mcs[1]],
                        in_=src[mcs[0]:mcs[1]].rearrange("mc p c -> p mc c"))

                def xload(dst, src, kc, eng):
                    # x 1-kc chunk (128KB: one DMA engine drains it in ~6us)
                    eng.dma_start(out=dst[:, kc, :], in_=src[ts(kc, 128), :])

                # x rides the HWDGE (shared device, ~0.63us/issue) in 1-kc
                # chunks so transfers spread across DMA engines: xq hi/lo
                # interleaved on SP, xkv hi on Act (lo deferred until after
                # the q-proj emit so it doesn't contend with xq). Weights
                # ride the Pool SWDGE queue (~1us fixed trigger each) in
                # chunks ordered to pace consumption: wqh per-mc first, then
                # wql / wmkh interleaved, wmkl, wmv.
                for kc in range(NKC):
                    xload(xqTh, xqh_d, kc, nc.sync)
                    xload(xqTl, xql_d, kc, nc.sync)
                for kc in range(NKC):
                    xload(xkvh, xkh_d, kc, nc.scalar)

                for mc in range(NKC):
                    wload(wqch, wqh_d, (mc, mc + 1))
                wload(wqcl, wql_d, (0, 1))
                wload(wmkh, wmkh_d, (0, 2))
                wload(wqcl, wql_d, (1, 2))
                wload(wqcl, wql_d, (2, 3))
                wload(wmkh, wmkh_d, (2, 4))
                wload(wqcl, wql_d, (3, 4))
                wload(wqcl, wql_d, (4, 5))
                wload(wmkh, wmkh_d, (4, 6))
                wload(wqcl, wql_d, (5, 6))
                wload(wqcl, wql_d, (6, 7))
                wload(wmkh, wmkh_d, (6, 8))
                wload(wqcl, wql_d, (7, 8))
                for mcp in range(4):
                    wload(wmkl, wmkl_d, (2 * mcp, 2 * mcp + 2))
                nc.gpsimd.dma_start(out=wmvh[:, 0:4], in_=wmvh_d[:, 0:4])
                nc.gpsimd.dma_start(out=wmvh[:, 4:8], in_=wmvh_d[:, 4:8])
                nc.gpsimd.dma_start(out=wmvl[:, 0:4], in_=wmvl_d[:, 0:4])
                nc.gpsimd.dma_start(out=wmvl[:, 4:8], in_=wmvl_d[:, 4:8])

                for jc in range(NJC):
                    nc.gpsimd.memset(vaug[:, jc, :, 64], 1.0)

                def dr_group(ps, wh, wl, xh, xl, wsl, xsl):
                    """12 DoubleRow matmuls: hh, hl, lh terms over 4 kc-pairs
                    (lh last so the W-lo DMA has the longest slack)."""
                    n = 0
                    for wt, xt in ((wh, xh), (wh, xl), (wl, xh)):
                        for kp in range(NKP):
                            nc.tensor.matmul(
                                ps, wsl(wt, kp), xsl(xt, kp),
                                start=(n == 0), stop=(n == 11), perf_mode=DR)
                            n += 1

                # q projection
                for mc in range(NKC):
                    ps = psum.tile([128, LQ], F32, tag="t1", bufs=2)
                    dr_group(
                        ps, wqch, wqcl, xqTh, xqTl,
                        lambda w, kp, mc=mc: w[:, mc, 2 * kp:2 * kp + 2, :],
                        lambda x, kp: x[:, 2 * kp:2 * kp + 2, :])
                    nc.scalar.activation(qT[:, mc, :], ps, CPY, scale=QSC)
                    if mc == 0:
                        # xkv lo loads issue on the Act queue after the first
                        # qT copy: keeps the HWDGE free for xq/xkv-hi early,
                        # lands in time for the k-proj hl terms
                        for kc in range(NKC):
                            xload(xkvl, xkl_d, kc, nc.scalar)

                # k projection
                for mc in range(NKC):
                    for nh in range(2):
                        ps = psum.tile([128, 512], F32, tag="t1", bufs=2)
                        dr_group(
                            ps, wmkh, wmkl, xkvh, xkvl,
                            lambda w, kp, mc=mc: w[:, mc, 2 * kp:2 * kp + 2, :],
                            lambda x, kp, nh=nh: x[:, 2 * kp:2 * kp + 2, ts(nh, 512)])
                        if nh == 0:
                            nc.vector.tensor_scalar_mul(kT[:, mc, ts(nh, 512)], ps, KSC)
                        else:
                            nc.scalar.activation(kT[:, mc, ts(nh, 512)], ps, CPY, scale=KSC)

                for h in range(4):
                    issue_srow(h)

                # v projection interleaved with pairs 0-1 scores/exp/bias-mul:
                # the Act-bound exp pipeline hides under v-proj PE work. The
                # attn matmuls for these pairs run after v-proj (deferred).
                vgroups = [(jc, nh) for jc in range(NJC) for nh in range(2)]
                vi = 0
                for c in range(2):
                    for hh in range(2):
                        h = 2 * c + hh
                        srow_sb = srow_tiles.pop(h)
                        for jp in range(NJC // 2):
                            scores_unit(c, hh, jp, srow_sb)
                            jcv, nh = vgroups[vi]
                            vi += 1
                            ps = psum.tile([128, 512], F32, tag="t1", bufs=2)
                            dr_group(
                                ps, xkvh, xkvl, wmvh, wmvl,
                                lambda x, kp, jcv=jcv: x[:, 2 * kp:2 * kp + 2, ts(jcv, 128)],
                                lambda w, kp, nh=nh: w[:, 2 * kp:2 * kp + 2, ts(nh, 512)])
                            dst = vaug[:, jcv, nh * 8:(nh + 1) * 8, 0:64]
                            if nh == 0:
                                nc.vector.tensor_scalar_mul(dst, ps, VSC)
                            else:
                                nc.scalar.activation(dst, ps, CPY, scale=VSC)
                        issue_srow(h + 4)

            # ================= phase C: attention =================
            with tc.tile_pool(name="phC", bufs=1) as phc:
                # gate-phase weights load during C on the Pool SWDGE queue
                # (it is idle between per-head lo-subtracts), chunked by kc
                # so the transfers spread across DMA engines, and issued one
                # pair per head block so they never form an issue convoy in
                # front of time-critical work.
                wgrh = phc.tile([128, NKC, 2 * D], FP8)
                wgrl = phc.tile([128, NKC, 2 * D], FP8)
                bg_sb = phc.tile([128, 2 * D], F32)
                nc.gpsimd.dma_start(out=bg_sb, in_=bg_d)

                wg_chunks = [(t, kc) for kc in range(NKC) for t in range(2)]

                def issue_wg(n=2):
                    for _ in range(n):
                        if wg_chunks:
                            t, kc = wg_chunks.pop(0)
                            dst, src = ((wgrh, wgh_d), (wgrl, wgl_d))[t]
                            nc.gpsimd.dma_start(out=dst[:, kc], in_=src[:, kc])

                def finish_head(c, hh, ps_h):
                    rsb = phc.tile([128, LQ], mdt, tag="rsb", bufs=2,
                                   name="rsb")
                    with nc.allow_low_precision(reason="softmax recip bf16"):
                        nc.vector.reciprocal(rsb[64:65, :], ps_h[64:65, :])
                    rb_ps = psum.tile([64, LQ], F32, tag="t1", bufs=2,
                                      name="rb_ps")
                    nc.tensor.matmul(
                        rb_ps, onesc[64:65, :], rsb[64:65, :],
                        start=True, stop=True)
                    # rb must bounce through SBUF (TensorTensor may read only
                    # one PSUM operand); the copy rides Act
                    rb_sb = phc.tile([64, LQ], F32, tag="rbs", bufs=2,
                                     name="rb_sb")
                    nc.scalar.activation(rb_sb, rb_ps, CPY)
                    tmp = phc.tile([64, LQ], mdt, tag="tmp", bufs=2, name="tmp")
                    nc.vector.tensor_tensor(tmp, ps_h[0:64, :], rb_sb, MUL)
                    with nc.allow_low_precision(reason="gate fp8 hi/lo"):
                        if hh == 0:
                            nc.vector.tensor_copy(attn_h[0:64, c, :], tmp)
                            nc.gpsimd.tensor_tensor(
                                attn_l[0:64, c, :], tmp, attn_h[0:64, c, :], SUB)
                        else:
                            thi = phc.tile([64, LQ], FP8, tag="thi", bufs=2,
                                           name="thi")
                            tlo = phc.tile([64, LQ], FP8, tag="tlo", bufs=2,
                                           name="tlo")
                            nc.vector.tensor_copy(thi, tmp)
                            nc.gpsimd.tensor_tensor(tlo, tmp, thi, SUB)
                            nc.sync.dma_start(out=attn_h[64:128, c, :], in_=thi)
                            nc.sync.dma_start(out=attn_l[64:128, c, :], in_=tlo)

                # deferred attn for the interleaved pairs
                for c in range(2):
                    for hh in range(2):
                        h = 2 * c + hh
                        issue_wg()
                        ps_h = psum.tile([65, LQ], F32, tag="psh", bufs=2)
                        for jc in range(NJC):
                            nc.tensor.matmul(
                                ps_h, vaug[:, jc, h, :], pending.pop((h, jc)),
                                start=(jc == 0), stop=(jc == NJC - 1))
                        finish_head(c, hh, ps_h)

                # remaining pairs: scores/exp/mul and attn pipelined with one
                # unit of lookahead so the attn matmuls never wait on the
                # exp -> bias-mul latency of their own unit
                for c in range(2, NKC):
                    for hh in range(2):
                        h = 2 * c + hh
                        srow_sb = srow_tiles.pop(h)
                        if h + 4 < H:
                            issue_srow(h + 4)
                        issue_wg()
                        ps_h = psum.tile([65, LQ], F32, tag="psh", bufs=2)
                        scores_unit(c, hh, 0, srow_sb)
                        for jp in range(NJC // 2):
                            if jp + 1 < NJC // 2:
                                scores_unit(c, hh, jp + 1, srow_sb)
                            for t in range(2):
                                jc = 2 * jp + t
                                nc.tensor.matmul(
                                    ps_h, vaug[:, jc, h, :],
                                    pending.pop((h, jc)),
                                    start=(jc == 0), stop=(jc == NJC - 1))
                        finish_head(c, hh, ps_h)

                # ================= phase D: gate =================
                def gate_dr(ps, ic, colsl):
                    # kp-major, kp3 (heads 12-15, finished last) at the end
                    n = 0
                    for kp in range(NKP):
                        for wt, xt in ((wgrh, attn_h), (wgrh, attn_l), (wgrl, attn_h)):
                            nc.tensor.matmul(
                                ps, xt[:, 2 * kp:2 * kp + 2, ts(ic, 128)],
                                wt[:, 2 * kp:2 * kp + 2, colsl],
                                start=(n == 0), stop=(n == 11), perf_mode=DR)
                            n += 1

                for ic in range(NIC):
                    for qa in range(2):
                        bsl = slice(D + qa * 512, D + qa * 512 + 512)
                        ps_b = psum.tile([128, 512], F32, tag="t1", bufs=2)
                        gate_dr(ps_b, ic, bsl)
                        tb = phc.tile([128, 512], F32, tag="tb", bufs=2)
                        nc.vector.scalar_tensor_tensor(
                            tb, ps_b, GSC, bg_sb[:, bsl], MUL, ADD)
                        tsg = phc.tile([128, 512], F32, tag="tsg", bufs=2)
                        nc.scalar.activation(tsg, tb, SIG)

                        asl = ts(qa, 512)
                        ps_a = psum.tile([128, 512], F32, tag="t1", bufs=2)
                        gate_dr(ps_a, ic, asl)
                        last = (ic == NIC - 1) and (qa == 1)
                        if not last:
                            ta = phc.tile([128, 512], F32, tag="ta", bufs=2)
                            nc.vector.scalar_tensor_tensor(
                                ta, ps_a, GSC, bg_sb[:, asl], MUL, ADD)
                            outh = phc.tile([128, 512], mdt, tag="outt", bufs=3)
                            nc.vector.tensor_tensor(outh, ta, tsg, MUL)
                            nc.sync.dma_start(
                                out=out_d[ts(ic, 128), asl], in_=outh)
                        else:
                            # final chunk in 128-col pieces, DMAs alternating
                            # SP/Act issue, for a short tail
                            for qt in range(4):
                                lo = qa * 512 + qt * 128
                                sl = slice(qt * 128, qt * 128 + 128)
                                ta = phc.tile([128, 128], F32, tag="ta2", bufs=2)
                                nc.vector.scalar_tensor_tensor(
                                    ta, ps_a[:, sl], GSC, bg_sb[:, lo:lo + 128],
                                    MUL, ADD)
                                outh = phc.tile([128, 128], mdt, tag="outt2", bufs=2)
                                nc.vector.tensor_tensor(outh, ta, tsg[:, sl], MUL)
                                eng = nc.sync if qt % 2 == 0 else nc.scalar
                                eng.dma_start(
                                    out=out_d[ts(ic, 128), lo:lo + 128], in_=outh)

    nc.compile()
    return nc


# ======================= host side =======================

def _tisa_ebias(amp, off, sharp):
    d = np.arange(-(L - 1), L, dtype=np.float32)
    s = np.sum(
        amp[:, :, None].astype(np.float32)
        * np.exp(-np.abs(sharp)[:, :, None].astype(np.float32)
                 * (d[None, None, :] - off[:, :, None].astype(np.float32)) ** 2),
        axis=1, dtype=np.float32).astype(np.float32)
    return np.exp(s).astype(np.float32)


def _hilo(x, s):
    """Split x*s into fp8 hi + lo (both e4m3)."""
    xs = (x * s).astype(np.float32)
    hi = xs.astype(NPF8)
    lo = (xs - hi.astype(np.float32)).astype(NPF8)
    return hi, lo


def make_host_inputs(inputs, cfg="fp8"):
    npdt = ml_dtypes.bfloat16
    x_q = np.asarray(inputs["x_q"])
    x_kv = np.asarray(inputs["x_kv"])
    wq = np.asarray(inputs["Wq"])
    wm = np.asarray(inputs["Wm"])
    wg = np.asarray(inputs["Wg"])
    bg = np.asarray(inputs["bg"]).astype(np.float32)

    ebias = _tisa_ebias(np.asarray(inputs["tisa_amp"]),
                        np.asarray(inputs["tisa_off"]),
                        np.asarray(inputs["tisa_sharp"]))

    p_i = np.arange(128)[:, None]
    m_i = np.arange(SROW_W)[None, :]
    srows = []
    for i_off in (0, 512):
        idx = p_i - m_i + (1919 - i_off)
        srows.append(np.ascontiguousarray(ebias[:, idx]).astype(npdt))

    def colchunk(w):
        # [k, m] -> [mc, p, kc*128]: [mc][p][kc*128+c] = w[kc*128+p, mc*128+c]
        return np.ascontiguousarray(
            w.reshape(NKC, 128, NKC, 128).transpose(2, 1, 0, 3).reshape(NKC, 128, D))

    def prow(w):
        # [k, cols] -> [p, kc, cols]
        return np.ascontiguousarray(
            w.reshape(NKC, 128, w.shape[1]).transpose(1, 0, 2))

    wqh, wql = _hilo(wq, SW)
    wmkh, wmkl = _hilo(wm[:, :D], SW)
    wmvh, wmvl = _hilo(wm[:, D:], SW)
    wgh, wgl = _hilo(wg, SW)

    bgrep = np.ascontiguousarray(np.broadcast_to(bg, (128, 2 * D))).astype(np.float32)

    in_maps = []
    for core in range(8):
        b, half = core // 2, core % 2
        xq = np.ascontiguousarray(x_q[b, half * LQ:(half + 1) * LQ].T)
        xkv = np.ascontiguousarray(x_kv[b].T)
        xqh, xql = _hilo(xq, SX)
        xkh, xkl = _hilo(xkv, SX)
        in_maps.append({
            "xqh": xqh, "xql": xql,
            "xkh": xkh, "xkl": xkl,
            "wqh": colchunk(wqh), "wql": colchunk(wql),
            "wmkh": colchunk(wmkh), "wmkl": colchunk(wmkl),
            "wmvh": prow(wmvh), "wmvl": prow(wmvl),
            "wgh": prow(wgh), "wgl": prow(wgl),
            "srow": srows[half],
            "bgrep": bgrep,
        })
    return in_maps


def assemble_output(results):
    out = np.empty((4, L, D), dtype=np.float32)
    for core in range(8):
        b, half = core // 2, core % 2
        out[b, half * LQ:(half + 1) * LQ] = results[core]["out"].astype(np.float32)
    return out


# ======================= public entry point =======================

_NC_CACHE = {}


def _get_nc(cfg):
    if cfg not in _NC_CACHE:
        _NC_CACHE[cfg] = build_nc(cfg)
    return _NC_CACHE[cfg]


def kernel(**inputs):
    """Full (unsharded) inputs -> full (4, 1024, 1024) float32 output.

    Shards over 8 NeuronCores: core = (batch, query-half). Host precomputes
    the TISA exp-bias lookup table, pre-transposes activations, and splits
    activations/weights into compensated fp8 hi/lo pairs; projections and
    gate run as fp8 DoubleRow matmuls, attention in bf16, all with fp32
    accumulation.
    """
    from concourse.bass_utils import run_bass_kernel_spmd

    cfg = "fp8"
    nc = _get_nc(cfg)
    in_maps = make_host_inputs(inputs, cfg)
    res = run_bass_kernel_spmd(nc, in_maps, core_ids=list(range(8)))
    return assemble_output(res.results)


# revision 45
# speedup vs baseline: 1.1488x; 1.0869x over previous
"""CrossAttn + TISA bias kernel for TRN2, 8-core SPMD.

Sharding: core = (batch b = core//2, query half = core%2).
Each core computes the full kv projection for its batch (duplicated within
the pair) and its 512 query rows end-to-end. No collectives.

v3: the four big GEMMs (q/k/v projections, gate) run as compensated fp8
DoubleRow matmuls. Each operand X is split hi/lo: xh = fp8(X*s),
xl = fp8(X*s - xh), and X@W is computed as xh@wh + xh@wl + xl@wh (the
lo*lo term is dropped; ~2^-8 relative error, bf16-level). DoubleRow packs
two 128-deep k-tiles per instruction at 0.5 PE cycles/row, so the 3-term
scheme costs 0.75x the bf16 GEMM at bf16-level precision.

  x scale 8 (max |x*8| ~ 42), W scale 1024 (max ~110), fp8e4m3 max 240.
  attn is rescaled by 32 through the v-projection descale so its hi/lo
  split (done on device from the bf16 normalize output) stays clear of
  the fp8 subnormal floor.

Scores and the attention matmul stay bf16: softmax weights cannot be
quantized to fp8 (3.6% rms error), and both matmuls are output-bound so
fp8 would not make them cheaper anyway.

Engine budget: PE ~140us (bottleneck), Act ~88 (exp-dominated), DVE ~90
(wT muls at the 2x bf16-SBUF rate), Pool ~50 (SWDGE triggers, lo-subs,
nh=0 copies). Weights are host-pre-arranged so every weight DMA is
contiguous (128 descriptors) - SWDGE trigger cost is ~1us fixed per DMA
on the Pool engine, so weight DMAs are few and big; x stays in 2-kc
chunks matching the DoubleRow pair granularity for early start.
"""

import numpy as np
import ml_dtypes

import concourse.bacc as bacc
import concourse.mybir as mybir
import concourse.tile as tile
from concourse.bass import ts

L = 1024
D = 1024
H = 16
DH = 64
LQ = 512          # q rows per core
NIC = LQ // 128   # 4 i-chunks
NJC = L // 128    # 8 j-chunks
NKC = D // 128    # 8 d_model chunks
NKP = NKC // 2    # 4 k-chunk pairs (DoubleRow)
SROW_W = 1408
NUM_KERNELS = 21

SX = 8.0          # x hi/lo fp8 scale
SW = 1024.0       # W hi/lo fp8 scale
SA = 32.0         # attn rescale for the gate's fp8 split
QSC = 0.125 / (SX * SW)   # q descale including 1/sqrt(DH)
KSC = 1.0 / (SX * SW)
VSC = SA / (SX * SW)
GSC = 1.0 / (SA * SW)     # gate psum descale

F32 = mybir.dt.float32
BF16 = mybir.dt.bfloat16
FP8 = mybir.dt.float8e4
DR = mybir.MatmulPerfMode.DoubleRow
EXP = mybir.ActivationFunctionType.Exp
SIG = mybir.ActivationFunctionType.Sigmoid
CPY = mybir.ActivationFunctionType.Copy
MUL = mybir.AluOpType.mult
ADD = mybir.AluOpType.add
SUB = mybir.AluOpType.subtract

NPF8 = ml_dtypes.float8_e4m3


def ds2(hh):
    return slice(hh * 64, hh * 64 + 64)


def build_nc(cfg="fp8"):
    mdt = BF16

    nc = bacc.Bacc("TRN2", target_bir_lowering=False, debug=False, num_devices=8)

    # x: [d_in, tokens] fp8 hi/lo
    xqh_d = nc.dram_tensor("xqh", [D, LQ], FP8, kind="ExternalInput").ap()
    xql_d = nc.dram_tensor("xql", [D, LQ], FP8, kind="ExternalInput").ap()
    xkh_d = nc.dram_tensor("xkh", [D, L], FP8, kind="ExternalInput").ap()
    xkl_d = nc.dram_tensor("xkl", [D, L], FP8, kind="ExternalInput").ap()
    # weights pre-arranged to SBUF layout (contiguous DMA):
    #   wq/wmk: [mc, p, kc*128]  (column-chunk-major, partition-contig)
    wqh_d = nc.dram_tensor("wqh", [NKC, 128, D], FP8, kind="ExternalInput").ap()
    wql_d = nc.dram_tensor("wql", [NKC, 128, D], FP8, kind="ExternalInput").ap()
    wmkh_d = nc.dram_tensor("wmkh", [NKC, 128, D], FP8, kind="ExternalInput").ap()
    wmkl_d = nc.dram_tensor("wmkl", [NKC, 128, D], FP8, kind="ExternalInput").ap()
    #   wmv/wg: [p, kc, cols]
    wmvh_d = nc.dram_tensor("wmvh", [128, NKC, D], FP8, kind="ExternalInput").ap()
    wmvl_d = nc.dram_tensor("wmvl", [128, NKC, D], FP8, kind="ExternalInput").ap()
    wgh_d = nc.dram_tensor("wgh", [128, NKC, 2 * D], FP8, kind="ExternalInput").ap()
    wgl_d = nc.dram_tensor("wgl", [128, NKC, 2 * D], FP8, kind="ExternalInput").ap()
    srow_d = nc.dram_tensor("srow", [H, 128, SROW_W], mdt, kind="ExternalInput").ap()
    bg_d = nc.dram_tensor("bgt", [128, 2 * NKC], F32, kind="ExternalInput").ap()
    id_d = nc.dram_tensor("ident", [128, 128], mdt, kind="ExternalInput").ap()
    out_d = nc.dram_tensor("out", [D, LQ], mdt, kind="ExternalOutput").ap()

    with tile.TileContext(nc) as tc:
        with (
            tc.tile_pool(name="const", bufs=1) as constp,
            tc.tile_pool(name="persist", bufs=1) as pers,
            tc.tile_pool(name="psum", bufs=1, space="PSUM") as psum,
        ):
            onesc = constp.tile([128, 64], mdt)
            nc.gpsimd.memset(onesc, 1.0)

            # PE warmup: keep the array busy during the initial input DMA so
            # the first real matmuls run at full clock (pstate ramp).
            dummy = constp.tile([128, 512], mdt)
            nc.gpsimd.memset(dummy, 0.0)
            for _ in range(14):
                ps_w = psum.tile([128, 512], F32, tag="t1", bufs=2)
                nc.tensor.matmul(ps_w, dummy[:, 0:128], dummy, start=True, stop=True)

            qT = pers.tile([128, NKC, LQ], mdt)        # [d_out, mc, i]
            kT = pers.tile([128, NKC, L], mdt)         # [d_out, mc, j]
            vaug = pers.tile([128, NJC, H, 65], mdt)   # [j, jc, h, v|1]
            attnT = pers.tile([128, NIC, H, 64], mdt)  # [i, ic, h, d] norm'd
            attn_h = pers.tile([128, NKC, LQ], FP8)    # fp8 hi of attn*SA
            attn_l = pers.tile([128, NKC, LQ], FP8)    # fp8 lo
            ident = constp.tile([128, 128], mdt)       # transpose permutation

            srow_tiles = {}

            def issue_srow(h):
                srow_tiles[h] = pers.tile(
                    [128, SROW_W], mdt, tag="srow", bufs=4, name=f"srow{h}")
                nc.sync.dma_start(out=srow_tiles[h], in_=srow_d[h, :, :])

            # scores + exp + bias-mul for one (head, jc-pair); wT tiles are
            # stashed in `pending` so the attn matmul can run later
            pending = {}

            def scores_unit(c, hh, jp, srow_sb):
                h = 2 * c + hh
                ps_s = psum.tile([128, 2 * LQ], F32, tag="ss", bufs=2,
                                 name="ps_s")
                for t in range(2):
                    jc = 2 * jp + t
                    nc.tensor.matmul(
                        ps_s[:, ts(t, LQ)],
                        kT[ds2(hh), c, ts(jc, 128)], qT[ds2(hh), c, :],
                        start=True, stop=True)
                wexp = pers.tile([128, 2 * LQ], mdt, tag="wexp", bufs=3,
                                 name="wexp")
                nc.scalar.activation(wexp, ps_s, EXP)
                for t in range(2):
                    jc = 2 * jp + t
                    wT = pers.tile([128, LQ], mdt, tag="wt", bufs=61,
                                   name="wT")
                    C0 = 896 - jc * 128
                    nc.vector.tensor_tensor(
                        wT, wexp[:, ts(t, LQ)], srow_sb[:, C0:C0 + LQ], MUL)
                    pending[(h, jc)] = wT

            # =========== phase B: projections ==========
            with tc.tile_pool(name="phB", bufs=1) as phb:
                wqch = phb.tile([128, NKC, NKC, 128], FP8)  # [k, mc, kc, col]
                wqcl = phb.tile([128, NKC, NKC, 128], FP8)
                xqTh = phb.tile([128, NKC, LQ], FP8)        # [d_in, kc, i]
                xqTl = phb.tile([128, NKC, LQ], FP8)
                xkvh = phb.tile([128, NKC, L], FP8)         # [d_in, kc, j]
                xkvl = phb.tile([128, NKC, L], FP8)
                wmkh = phb.tile([128, NKC, NKC, 128], FP8)
                wmkl = phb.tile([128, NKC, NKC, 128], FP8)
                wmvh = phb.tile([128, NKC, D], FP8)         # [k, kc, v-cols]
                wmvl = phb.tile([128, NKC, D], FP8)

                def wload(dst, src, mcs):
                    # weight chunk-group load: [n_mc, 128, D] -> [128, n_mc, D]
                    nc.gpsimd.dma_start(
                        out=dst[:, mcs[0]:mcs[1]],
                        in_=src[mcs[0]:mcs[1]].rearrange("mc p c -> p mc c"))

                def xload(dst, src, kc, eng):
                    # x 1-kc chunk (128KB: one DMA engine drains it in ~6us)
                    eng.dma_start(out=dst[:, kc, :], in_=src[ts(kc, 128), :])

                # x rides the HWDGE (shared device, ~0.63us/issue) in 1-kc
                # chunks so transfers spread across DMA engines: xq hi/lo
                # interleaved on SP, xkv hi on Act (lo deferred until after
                # the q-proj emit so it doesn't contend with xq). Weights
                # ride the Pool SWDGE queue (~1us fixed trigger each) in
                # chunks ordered to pace consumption: wqh per-mc first, then
                # wql / wmkh interleaved, wmkl, wmv.
                for kc in range(NKC):
                    xload(xqTh, xqh_d, kc, nc.sync)
                    xload(xqTl, xql_d, kc, nc.sync)
                for kc in range(NKC):
                    xload(xkvh, xkh_d, kc, nc.scalar)

                for mc in range(NKC):
                    wload(wqch, wqh_d, (mc, mc + 1))
                wload(wqcl, wql_d, (0, 1))
                wload(wmkh, wmkh_d, (0, 2))
                wload(wqcl, wql_d, (1, 2))
                wload(wqcl, wql_d, (2, 3))
                wload(wmkh, wmkh_d, (2, 4))
                wload(wqcl, wql_d, (3, 4))
                wload(wqcl, wql_d, (4, 5))
                wload(wmkh, wmkh_d, (4, 6))
                wload(wqcl, wql_d, (5, 6))
                wload(wqcl, wql_d, (6, 7))
                wload(wmkh, wmkh_d, (6, 8))
                wload(wqcl, wql_d, (7, 8))
                for mcp in range(4):
                    wload(wmkl, wmkl_d, (2 * mcp, 2 * mcp + 2))
                nc.gpsimd.dma_start(out=wmvh[:, 0:4], in_=wmvh_d[:, 0:4])
                nc.gpsimd.dma_start(out=wmvh[:, 4:8], in_=wmvh_d[:, 4:8])
                nc.gpsimd.dma_start(out=wmvl[:, 0:4], in_=wmvl_d[:, 0:4])
                nc.gpsimd.dma_start(out=wmvl[:, 4:8], in_=wmvl_d[:, 4:8])

                for jc in range(NJC):
                    nc.gpsimd.memset(vaug[:, jc, :, 64], 1.0)

                def dr_group(ps, wh, wl, xh, xl, wsl, xsl):
                    """12 DoubleRow matmuls: hh, hl, lh terms over 4 kc-pairs
                    (lh last so the W-lo DMA has the longest slack)."""
                    n = 0
                    for wt, xt in ((wh, xh), (wh, xl), (wl, xh)):
                        for kp in range(NKP):
                            nc.tensor.matmul(
                                ps, wsl(wt, kp), xsl(xt, kp),
                                start=(n == 0), stop=(n == 11), perf_mode=DR)
                            n += 1

                # q/k projections run as "waves" of up to 6 concurrently-open
                # PSUM accumulation groups (2 ss pair-tiles give 4 halves +
                # 2 t1 slots), emitted term/kp-major across the wave: each
                # arriving x chunk or weight chunk unlocks one DR matmul in
                # EVERY open group, so the PE rides the DMA arrival front
                # instead of serializing two groups at a time.
                def wave_slots(n):
                    slots = []
                    i = 0
                    while i + 1 < min(n, 5):
                        pp = psum.tile([128, 2 * LQ], F32, tag="ss", bufs=2,
                                       name="wave_pair")
                        slots += [pp[:, 0:512], pp[:, 512:1024]]
                        i += 2
                    while i < n:
                        ps_w1 = psum.tile([128, LQ], F32, tag="t1", bufs=2,
                                          name="wave_t1")
                        slots.append(ps_w1)
                        i += 1
                    return slots

                def dr_wave(groups, wh, wl, xh, xl, wsl, xsl, finish):
                    slots = wave_slots(len(groups))
                    nt = 0
                    for wt, xt in ((wh, xh), (wh, xl), (wl, xh)):
                        for kp in range(NKP):
                            for ps, g in zip(slots, groups):
                                nc.tensor.matmul(
                                    ps, wsl(wt, kp, g), xsl(xt, kp, g),
                                    start=(nt == 0), stop=(nt == 11),
                                    perf_mode=DR)
                            nt += 1
                    for ps, g in zip(slots, groups):
                        finish(ps, g)

                def q_finish(ps, mc):
                    nc.scalar.activation(qT[:, mc, :], ps, CPY, scale=QSC)

                dr_wave(
                    list(range(6)), wqch, wqcl, xqTh, xqTl,
                    lambda w, kp, mc: w[:, mc, 2 * kp:2 * kp + 2, :],
                    lambda x, kp, mc: x[:, 2 * kp:2 * kp + 2, :],
                    q_finish)
                # xkv lo loads issue on the Act queue mid-q-proj: keeps the
                # HWDGE free for xq/xkv-hi early, lands before k-proj hl terms
                for kc in range(NKC):
                    xload(xkvl, xkl_d, kc, nc.scalar)
                dr_wave(
                    [6, 7], wqch, wqcl, xqTh, xqTl,
                    lambda w, kp, mc: w[:, mc, 2 * kp:2 * kp + 2, :],
                    lambda x, kp, mc: x[:, 2 * kp:2 * kp + 2, :],
                    q_finish)

                # k projection
                def k_finish(ps, g):
                    mc, nh = g
                    nc.vector.tensor_scalar_mul(kT[:, mc, ts(nh, 512)], ps, KSC)

                kgroups = [(mc, nh) for mc in range(NKC) for nh in range(2)]
                for w0 in (0, 6, 12):
                    dr_wave(
                        kgroups[w0:w0 + 6], wmkh, wmkl, xkvh, xkvl,
                        lambda w, kp, g: w[:, g[0], 2 * kp:2 * kp + 2, :],
                        lambda x, kp, g: x[:, 2 * kp:2 * kp + 2, ts(g[1], 512)],
                        k_finish)

                for h in range(4):
                    issue_srow(h)
                nc.sync.dma_start(out=ident, in_=id_d)

                # v projection interleaved with heads 0-7's scores/exp/
                # bias-mul (two scores units per v-group): front-loading 8
                # heads of the Act-bound exp pipeline into phase B keeps Act
                # from pacing phase C. The attn matmuls for these heads run
                # in C (at_queue).
                vgroups = [(jc, nh) for jc in range(NJC) for nh in range(2)]
                vi = 0
                for c in range(4):
                    for hh in range(2):
                        h = 2 * c + hh
                        srow_sb = srow_tiles.pop(h)
                        issue_srow(h + 4)
                        for jp in range(NJC // 2):
                            scores_unit(c, hh, jp, srow_sb)
                            if jp % 2 == 1:
                                jcv, nh = vgroups[vi]
                                vi += 1
                                ps = psum.tile([128, 512], F32, tag="t1", bufs=2)
                                dr_group(
                                    ps, xkvh, xkvl, wmvh, wmvl,
                                    lambda x, kp, jcv=jcv: x[:, 2 * kp:2 * kp + 2, ts(jcv, 128)],
                                    lambda w, kp, nh=nh: w[:, 2 * kp:2 * kp + 2, ts(nh, 512)])
                                dst = vaug[:, jcv, nh * 8:(nh + 1) * 8, 0:64]
                                nc.vector.tensor_scalar_mul(dst, ps, VSC)

            # ================= phase C: attention =================
            with tc.tile_pool(name="phC", bufs=1) as phc:
                # gate-phase weights load during C on the Pool SWDGE queue
                # (it is idle between per-head lo-subtracts), chunked by kc
                # so the transfers spread across DMA engines, and issued one
                # pair per head block so they never form an issue convoy in
                # front of time-critical work.
                wgrh = phc.tile([128, NKC, 2 * D], FP8)
                wgrl = phc.tile([128, NKC, 2 * D], FP8)
                bgt_sb = phc.tile([128, 2 * NKC], F32)
                nc.gpsimd.dma_start(out=bgt_sb, in_=bg_d)

                wg_chunks = [(t, kc) for kc in range(NKC) for t in range(2)]

                def issue_wg(n=2):
                    for _ in range(n):
                        if wg_chunks:
                            t, kc = wg_chunks.pop(0)
                            dst, src = ((wgrh, wgh_d), (wgrl, wgl_d))[t]
                            nc.gpsimd.dma_start(out=dst[:, kc], in_=src[:, kc])

                # Attention runs TRANSPOSED: out[i, hd] with wT as the
                # stationary and vaug [128j, 65] as the moving tensor, so
                # each accumulation matmul costs 65 rows instead of 512
                # (the matmul cost is the moving free size). Bonuses: the
                # softmax denominator lands as a per-partition column, so
                # normalization is a single tensor_scalar with a [128,1]
                # scalar AP (no ones-broadcast matmul, no rb copies), and a
                # paired PE transpose rebuilds the [d, i] layout the gate
                # needs - writing even heads to partitions 0-63 and odd to
                # 64-127 directly (no cross-partition DMA).
                def at_group(c, hh, ic):
                    h = 2 * c + hh
                    ps_at = psum.tile([128, 65], F32, tag="at", bufs=2,
                                      name="ps_at")
                    for jc in range(NJC):
                        nc.tensor.matmul(
                            ps_at, pending[(h, jc)][:, ts(ic, 128)],
                            vaug[:, jc, h, :],
                            start=(jc == 0), stop=(jc == NJC - 1))
                    rsb = phc.tile([128, 1], F32, tag="rsb", bufs=4,
                                   name="rsb")
                    with nc.allow_low_precision(reason="softmax recip"):
                        nc.vector.reciprocal(rsb, ps_at[:, 64:65])
                        nc.vector.tensor_scalar_mul(
                            attnT[:, ic, h, :], ps_at[:, 0:64], rsb)
                    if ic == NIC - 1:
                        for jc in range(NJC):
                            pending.pop((h, jc))

                def finish_pair(c):
                    # transpose both heads' normalized attn back to [d, i]
                    # and split hi/lo fp8 for the gate
                    for ic in range(NIC):
                        pt = psum.tile([128, 128], mdt, tag="t1", bufs=2,
                                       name="pt")
                        nc.tensor.transpose(pt, attnT[:, ic, 2 * c:2 * c + 2, :], ident)
                        with nc.allow_low_precision(reason="gate fp8 hi/lo"):
                            nc.scalar.activation(attn_h[:, c, ts(ic, 128)], pt,
                                                 CPY)
                            nc.vector.tensor_tensor(
                                attn_l[:, c, ts(ic, 128)], pt,
                                attn_h[:, c, ts(ic, 128)], SUB)

                # pending attn-T groups: emitted one per scores_unit so each
                # head's attn matmuls interleave with the NEXT head's
                # scores/exp pipeline
                at_queue = []

                def emit_at(n):
                    for _ in range(n):
                        if not at_queue:
                            return
                        c0, hh0, ic0 = at_queue.pop(0)
                        at_group(c0, hh0, ic0)
                        if hh0 == 1 and ic0 == NIC - 1:
                            finish_pair(c0)

                # the interleaved heads' wT are ready; queue their groups
                for c in range(4):
                    for hh in range(2):
                        at_queue += [(c, hh, ic) for ic in range(NIC)]

                for c in range(4, NKC):
                    for hh in range(2):
                        h = 2 * c + hh
                        srow_sb = srow_tiles.pop(h)
                        if h + 4 < H:
                            issue_srow(h + 4)
                        issue_wg()
                        for jp in range(NJC // 2):
                            scores_unit(c, hh, jp, srow_sb)
                            emit_at(2)
                        at_queue += [(c, hh, ic) for ic in range(NIC)]
                emit_at(len(at_queue))

                # ================= phase D: gate (transposed) =================
                # out[cols, i] = Wg[:, cols].T @ attn: with gate columns on
                # PARTITIONS the bias is per-partition, so the sigmoid fuses
                # scale+bias into the Act instruction, the a-half fuses them
                # into one DVE tensor_scalar, and the host un-transposes the
                # [D, LQ] output for free.
                def gate_drT(ps, colsl):
                    # kp-major, kp3 (heads 12-15, finished last) at the end
                    n = 0
                    for kp in range(NKP):
                        for wt, xt in ((wgrh, attn_h), (wgrh, attn_l), (wgrl, attn_h)):
                            nc.tensor.matmul(
                                ps, wt[:, 2 * kp:2 * kp + 2, colsl],
                                xt[:, 2 * kp:2 * kp + 2, :],
                                start=(n == 0), stop=(n == 11), perf_mode=DR)
                            n += 1

                for cc in range(NKC):
                    bsl = slice(D + cc * 128, D + cc * 128 + 128)
                    ps_b = psum.tile([128, 512], F32, tag="t1", bufs=2)
                    gate_drT(ps_b, bsl)
                    tsg = phc.tile([128, 512], F32, tag="tsg", bufs=2)
                    nc.scalar.activation(tsg, ps_b, SIG, scale=GSC,
                                         bias=bgt_sb[:, 8 + cc:9 + cc])

                    asl = slice(cc * 128, cc * 128 + 128)
                    ps_a = psum.tile([128, 512], F32, tag="t1", bufs=2)
                    gate_drT(ps_a, asl)
                    if cc < NKC - 1:
                        ta = phc.tile([128, 512], F32, tag="ta", bufs=2)
                        nc.vector.tensor_scalar(
                            ta, ps_a, GSC, bgt_sb[:, cc:cc + 1], MUL, ADD)
                        outh = phc.tile([128, 512], mdt, tag="outt", bufs=3)
                        nc.vector.tensor_tensor(outh, ta, tsg, MUL)
                        nc.sync.dma_start(out=out_d[ts(cc, 128), :], in_=outh)
                    else:
                        # final col-chunk in 128-token pieces, DMAs
                        # alternating SP/Act issue, for a short tail
                        for qt in range(4):
                            sl = slice(qt * 128, qt * 128 + 128)
                            ta = phc.tile([128, 128], F32, tag="ta2", bufs=2)
                            nc.vector.tensor_scalar(
                                ta, ps_a[:, sl], GSC, bgt_sb[:, cc:cc + 1],
                                MUL, ADD)
                            outh = phc.tile([128, 128], mdt, tag="outt2", bufs=2)
                            nc.vector.tensor_tensor(outh, ta, tsg[:, sl], MUL)
                            eng = nc.sync if qt % 2 == 0 else nc.scalar
                            eng.dma_start(out=out_d[ts(cc, 128), sl], in_=outh)

    nc.compile()
    return nc


# ======================= host side =======================

def _tisa_ebias(amp, off, sharp):
    d = np.arange(-(L - 1), L, dtype=np.float32)
    s = np.sum(
        amp[:, :, None].astype(np.float32)
        * np.exp(-np.abs(sharp)[:, :, None].astype(np.float32)
                 * (d[None, None, :] - off[:, :, None].astype(np.float32)) ** 2),
        axis=1, dtype=np.float32).astype(np.float32)
    return np.exp(s).astype(np.float32)


def _hilo(x, s):
    """Split x*s into fp8 hi + lo (both e4m3)."""
    xs = (x * s).astype(np.float32)
    hi = xs.astype(NPF8)
    lo = (xs - hi.astype(np.float32)).astype(NPF8)
    return hi, lo


def make_host_inputs(inputs, cfg="fp8"):
    npdt = ml_dtypes.bfloat16
    x_q = np.asarray(inputs["x_q"])
    x_kv = np.asarray(inputs["x_kv"])
    wq = np.asarray(inputs["Wq"])
    wm = np.asarray(inputs["Wm"])
    wg = np.asarray(inputs["Wg"])
    bg = np.asarray(inputs["bg"]).astype(np.float32)

    ebias = _tisa_ebias(np.asarray(inputs["tisa_amp"]),
                        np.asarray(inputs["tisa_off"]),
                        np.asarray(inputs["tisa_sharp"]))

    p_i = np.arange(128)[:, None]
    m_i = np.arange(SROW_W)[None, :]
    srows = []
    for i_off in (0, 512):
        idx = p_i - m_i + (1919 - i_off)
        srows.append(np.ascontiguousarray(ebias[:, idx]).astype(npdt))

    def colchunk(w):
        # [k, m] -> [mc, p, kc*128]: [mc][p][kc*128+c] = w[kc*128+p, mc*128+c]
        return np.ascontiguousarray(
            w.reshape(NKC, 128, NKC, 128).transpose(2, 1, 0, 3).reshape(NKC, 128, D))

    def prow(w):
        # [k, cols] -> [p, kc, cols]
        return np.ascontiguousarray(
            w.reshape(NKC, 128, w.shape[1]).transpose(1, 0, 2))

    wqh, wql = _hilo(wq, SW)
    wmkh, wmkl = _hilo(wm[:, :D], SW)
    wmvh, wmvl = _hilo(wm[:, D:], SW)
    wgh, wgl = _hilo(wg, SW)

    bgt = np.ascontiguousarray(bg.reshape(2 * NKC, 128).T).astype(np.float32)

    in_maps = []
    for core in range(8):
        b, half = core // 2, core % 2
        xq = np.ascontiguousarray(x_q[b, half * LQ:(half + 1) * LQ].T)
        xkv = np.ascontiguousarray(x_kv[b].T)
        xqh, xql = _hilo(xq, SX)
        xkh, xkl = _hilo(xkv, SX)
        in_maps.append({
            "xqh": xqh, "xql": xql,
            "xkh": xkh, "xkl": xkl,
            "wqh": colchunk(wqh), "wql": colchunk(wql),
            "wmkh": colchunk(wmkh), "wmkl": colchunk(wmkl),
            "wmvh": prow(wmvh), "wmvl": prow(wmvl),
            "wgh": prow(wgh), "wgl": prow(wgl),
            "srow": srows[half],
            "bgt": bgt,
            "ident": np.eye(128, dtype=npdt),
        })
    return in_maps


def assemble_output(results):
    out = np.empty((4, L, D), dtype=np.float32)
    for core in range(8):
        b, half = core // 2, core % 2
        out[b, half * LQ:(half + 1) * LQ] = results[core]["out"].astype(np.float32).T
    return out


# ======================= public entry point =======================

_NC_CACHE = {}


def _get_nc(cfg):
    if cfg not in _NC_CACHE:
        _NC_CACHE[cfg] = build_nc(cfg)
    return _NC_CACHE[cfg]


def kernel(**inputs):
    """Full (unsharded) inputs -> full (4, 1024, 1024) float32 output.

    Shards over 8 NeuronCores: core = (batch, query-half). Host precomputes
    the TISA exp-bias lookup table, pre-transposes activations, and splits
    activations/weights into compensated fp8 hi/lo pairs; projections and
    gate run as fp8 DoubleRow matmuls, attention in bf16, all with fp32
    accumulation.
    """
    from concourse.bass_utils import run_bass_kernel_spmd

    cfg = "fp8"
    nc = _get_nc(cfg)
    in_maps = make_host_inputs(inputs, cfg)
    res = run_bass_kernel_spmd(nc, in_maps, core_ids=list(range(8)))
    return assemble_output(res.results)


# revision 59
# speedup vs baseline: 1.1602x; 1.0099x over previous
"""CrossAttn + TISA bias kernel for TRN2, 8-core SPMD.

Sharding: core = (batch b = core//2, query half = core%2).
Each core computes the full kv projection for its batch (duplicated within
the pair) and its 512 query rows end-to-end. No collectives.

v4 design (192us baseline -> ~165us cost model):

1. Compensated fp8 DoubleRow GEMMs (q/k/v projections, gate). Each
   operand X is split hi/lo: xh = fp8(X*s), xl = fp8(X*s - xh), and
   X@W = xh@wh + xh@wl + xl@wh (dropping lo*lo leaves ~2^-8 relative
   error, bf16-level). DoubleRow packs two 128-deep k-tiles per
   instruction at 0.5 PE cycles/row, so the 3-term scheme costs 0.75x
   the bf16 GEMM. x scale 8 (max ~42), W scale 1024 (max ~110), fp8e4m3
   max 240. attn is rescaled by 32 through the v-projection descale so
   its on-device hi/lo split clears the fp8 subnormal floor.

2. Scores stay bf16 (softmax weights cannot be fp8: 3.6% rms error, and
   the scores matmul is output-bound anyway). The attention matmul runs
   TRANSPOSED - out[i, hd] with wT stationary and vaug [128j, v|1]
   moving - so each accumulation matmul costs 65 rows instead of 512.
   The softmax denominator lands as a per-partition column (normalize =
   one tensor_scalar with a [128,1] scalar AP; no ones-broadcast
   matmul), and a paired PE transpose (identity permutation) rebuilds
   [d, i] with even heads on partitions 0-63 and odd on 64-127.

3. The gate also runs transposed (out[cols, i]): the gate bias becomes
   per-partition and fuses into the sigmoid activation; the host
   un-transposes the [D, LQ] bf16 output for free.

4. Scheduling: q/k projections run as 6-wide column-major PSUM waves so
   every arriving DMA chunk unlocks a matmul in each open group; 8 of
   16 heads' scores/exp pipelines are front-loaded into phase B under
   the v-projection; weight DMAs are host-pre-arranged contiguous
   chunks on the Pool SWDGE queue (~1us fixed trigger each), x rides
   the HWDGE per-kc; the last gate chunk drains through 128-col pieces
   on alternating DMA queues.

Engine busy (cost model): PE ~121us (bottleneck), Act ~98 (exp),
DVE ~94 (wT muls at the 2x bf16-SBUF rate), Pool ~49.
"""

import numpy as np
import ml_dtypes

import concourse.bacc as bacc
import concourse.mybir as mybir
import concourse.tile as tile
from concourse.bass import ts

L = 1024
D = 1024
H = 16
DH = 64
LQ = 512          # q rows per core
NIC = LQ // 128   # 4 i-chunks
NJC = L // 128    # 8 j-chunks
NKC = D // 128    # 8 d_model chunks
NKP = NKC // 2    # 4 k-chunk pairs (DoubleRow)
SROW_W = 1408
NUM_KERNELS = 21

SX = 8.0          # x hi/lo fp8 scale
SW = 1024.0       # W hi/lo fp8 scale
SA = 32.0         # attn rescale for the gate's fp8 split
QSC = 0.125 / (SX * SW)   # q descale including 1/sqrt(DH)
KSC = 1.0 / (SX * SW)
VSC = SA / (SX * SW)
GSC = 1.0 / (SA * SW)     # gate psum descale

F32 = mybir.dt.float32
BF16 = mybir.dt.bfloat16
FP8 = mybir.dt.float8e4
DR = mybir.MatmulPerfMode.DoubleRow
EXP = mybir.ActivationFunctionType.Exp
SIG = mybir.ActivationFunctionType.Sigmoid
CPY = mybir.ActivationFunctionType.Copy
MUL = mybir.AluOpType.mult
ADD = mybir.AluOpType.add
SUB = mybir.AluOpType.subtract

NPF8 = ml_dtypes.float8_e4m3


def ds2(hh):
    return slice(hh * 64, hh * 64 + 64)


def build_nc(cfg="fp8"):
    mdt = BF16

    nc = bacc.Bacc("TRN2", target_bir_lowering=False, debug=False, num_devices=8)

    # x: [d_in, tokens] fp8 hi/lo
    xqh_d = nc.dram_tensor("xqh", [D, LQ], FP8, kind="ExternalInput").ap()
    xql_d = nc.dram_tensor("xql", [D, LQ], FP8, kind="ExternalInput").ap()
    xkh_d = nc.dram_tensor("xkh", [D, L], FP8, kind="ExternalInput").ap()
    xkl_d = nc.dram_tensor("xkl", [D, L], FP8, kind="ExternalInput").ap()
    # weights pre-arranged to SBUF layout (contiguous DMA):
    #   wq/wmk: [mc, p, kc*128]  (column-chunk-major, partition-contig)
    wqh_d = nc.dram_tensor("wqh", [NKC, 128, D], FP8, kind="ExternalInput").ap()
    wql_d = nc.dram_tensor("wql", [NKC, 128, D], FP8, kind="ExternalInput").ap()
    wmkh_d = nc.dram_tensor("wmkh", [NKC, 128, D], FP8, kind="ExternalInput").ap()
    wmkl_d = nc.dram_tensor("wmkl", [NKC, 128, D], FP8, kind="ExternalInput").ap()
    #   wmv/wg: [p, kc, cols]
    wmvh_d = nc.dram_tensor("wmvh", [128, NKC, D], FP8, kind="ExternalInput").ap()
    wmvl_d = nc.dram_tensor("wmvl", [128, NKC, D], FP8, kind="ExternalInput").ap()
    wgh_d = nc.dram_tensor("wgh", [128, NKC, 2 * D], FP8, kind="ExternalInput").ap()
    wgl_d = nc.dram_tensor("wgl", [128, NKC, 2 * D], FP8, kind="ExternalInput").ap()
    srow_d = nc.dram_tensor("srow", [H, 128, SROW_W], mdt, kind="ExternalInput").ap()
    bg_d = nc.dram_tensor("bgt", [128, 2 * NKC], F32, kind="ExternalInput").ap()
    id_d = nc.dram_tensor("ident", [128, 128], mdt, kind="ExternalInput").ap()
    out_d = nc.dram_tensor("out", [D, LQ], mdt, kind="ExternalOutput").ap()

    with tile.TileContext(nc) as tc:
        with (
            tc.tile_pool(name="const", bufs=1) as constp,
            tc.tile_pool(name="persist", bufs=1) as pers,
            tc.tile_pool(name="psum", bufs=1, space="PSUM") as psum,
        ):
            onesc = constp.tile([128, 64], mdt)
            nc.gpsimd.memset(onesc, 1.0)

            # PE warmup: keep the array busy during the initial input DMA so
            # the first real matmuls run at full clock (pstate ramp).
            dummy = constp.tile([128, 512], mdt)
            nc.gpsimd.memset(dummy, 0.0)
            for _ in range(14):
                ps_w = psum.tile([128, 512], F32, tag="t1", bufs=2)
                nc.tensor.matmul(ps_w, dummy[:, 0:128], dummy, start=True, stop=True)

            qT = pers.tile([128, NKC, LQ], mdt)        # [d_out, mc, i]
            kT = pers.tile([128, NKC, L], mdt)         # [d_out, mc, j]
            vaug = pers.tile([128, NJC, H, 65], mdt)   # [j, jc, h, v|1]
            attnT = pers.tile([128, NIC, H, 64], mdt)  # [i, ic, h, d] norm'd
            attn_h = pers.tile([128, NKC, LQ], FP8)    # fp8 hi of attn*SA
            attn_l = pers.tile([128, NKC, LQ], FP8)    # fp8 lo
            ident = constp.tile([128, 128], mdt)       # transpose permutation

            srow_tiles = {}

            def issue_srow(h):
                srow_tiles[h] = pers.tile(
                    [128, SROW_W], mdt, tag="srow", bufs=4, name=f"srow{h}")
                nc.sync.dma_start(out=srow_tiles[h], in_=srow_d[h, :, :])

            # scores + exp + bias-mul for one (head, jc-pair); wT tiles are
            # stashed in `pending` so the attn matmul can run later
            pending = {}

            def scores_unit(c, hh, jp, srow_sb):
                h = 2 * c + hh
                ps_s = psum.tile([128, 2 * LQ], F32, tag="ss", bufs=2,
                                 name="ps_s")
                for t in range(2):
                    jc = 2 * jp + t
                    nc.tensor.matmul(
                        ps_s[:, ts(t, LQ)],
                        kT[ds2(hh), c, ts(jc, 128)], qT[ds2(hh), c, :],
                        start=True, stop=True)
                wexp = pers.tile([128, 2 * LQ], mdt, tag="wexp", bufs=3,
                                 name="wexp")
                nc.scalar.activation(wexp, ps_s, EXP)
                for t in range(2):
                    jc = 2 * jp + t
                    wT = pers.tile([128, LQ], mdt, tag="wt", bufs=61,
                                   name="wT")
                    C0 = 896 - jc * 128
                    nc.vector.tensor_tensor(
                        wT, wexp[:, ts(t, LQ)], srow_sb[:, C0:C0 + LQ], MUL)
                    pending[(h, jc)] = wT

            # =========== phase B: projections ==========
            with tc.tile_pool(name="phB", bufs=1) as phb:
                wqch = phb.tile([128, NKC, NKC, 128], FP8)  # [k, mc, kc, col]
                wqcl = phb.tile([128, NKC, NKC, 128], FP8)
                xqTh = phb.tile([128, NKC, LQ], FP8)        # [d_in, kc, i]
                xqTl = phb.tile([128, NKC, LQ], FP8)
                xkvh = phb.tile([128, NKC, L], FP8)         # [d_in, kc, j]
                xkvl = phb.tile([128, NKC, L], FP8)
                wmkh = phb.tile([128, NKC, NKC, 128], FP8)
                wmkl = phb.tile([128, NKC, NKC, 128], FP8)
                wmvh = phb.tile([128, NKC, D], FP8)         # [k, kc, v-cols]
                wmvl = phb.tile([128, NKC, D], FP8)

                def wload(dst, src, mcs):
                    # weight chunk-group load: [n_mc, 128, D] -> [128, n_mc, D]
                    nc.gpsimd.dma_start(
                        out=dst[:, mcs[0]:mcs[1]],
                        in_=src[mcs[0]:mcs[1]].rearrange("mc p c -> p mc c"))

                def xload(dst, src, kc, eng):
                    # x 1-kc chunk (128KB: one DMA engine drains it in ~6us)
                    eng.dma_start(out=dst[:, kc, :], in_=src[ts(kc, 128), :])

                # x rides the HWDGE (shared device, ~0.63us/issue) in 1-kc
                # chunks so transfers spread across DMA engines: xq hi/lo
                # interleaved on SP, xkv hi on Act (lo deferred until after
                # the q-proj emit so it doesn't contend with xq). Weights
                # ride the Pool SWDGE queue (~1us fixed trigger each) in
                # chunks ordered to pace consumption: wqh per-mc first, then
                # wql / wmkh interleaved, wmkl, wmv.
                for kc in range(NKC):
                    xload(xqTh, xqh_d, kc, nc.sync)
                    xload(xqTl, xql_d, kc, nc.sync)
                for kc in range(NKC):
                    xload(xkvh, xkh_d, kc, nc.scalar)

                for mc in range(NKC):
                    wload(wqch, wqh_d, (mc, mc + 1))
                wload(wqcl, wql_d, (0, 1))
                wload(wmkh, wmkh_d, (0, 2))
                wload(wqcl, wql_d, (1, 2))
                wload(wqcl, wql_d, (2, 3))
                wload(wmkh, wmkh_d, (2, 4))
                wload(wqcl, wql_d, (3, 4))
                wload(wqcl, wql_d, (4, 5))
                wload(wmkh, wmkh_d, (4, 6))
                wload(wqcl, wql_d, (5, 6))
                wload(wqcl, wql_d, (6, 7))
                wload(wmkh, wmkh_d, (6, 8))
                wload(wqcl, wql_d, (7, 8))
                for mcp in range(4):
                    wload(wmkl, wmkl_d, (2 * mcp, 2 * mcp + 2))
                nc.gpsimd.dma_start(out=wmvh[:, 0:4], in_=wmvh_d[:, 0:4])
                nc.gpsimd.dma_start(out=wmvh[:, 4:8], in_=wmvh_d[:, 4:8])
                nc.gpsimd.dma_start(out=wmvl[:, 0:4], in_=wmvl_d[:, 0:4])
                nc.gpsimd.dma_start(out=wmvl[:, 4:8], in_=wmvl_d[:, 4:8])

                for jc in range(NJC):
                    nc.gpsimd.memset(vaug[:, jc, :, 64], 1.0)

                def dr_group(ps, wh, wl, xh, xl, wsl, xsl):
                    """12 DoubleRow matmuls: hh, hl, lh terms over 4 kc-pairs
                    (lh last so the W-lo DMA has the longest slack)."""
                    n = 0
                    for wt, xt in ((wh, xh), (wh, xl), (wl, xh)):
                        for kp in range(NKP):
                            nc.tensor.matmul(
                                ps, wsl(wt, kp), xsl(xt, kp),
                                start=(n == 0), stop=(n == 11), perf_mode=DR)
                            n += 1

                # q/k projections run as "waves" of up to 6 concurrently-open
                # PSUM accumulation groups (2 ss pair-tiles give 4 halves +
                # 2 t1 slots), emitted term/kp-major across the wave: each
                # arriving x chunk or weight chunk unlocks one DR matmul in
                # EVERY open group, so the PE rides the DMA arrival front
                # instead of serializing two groups at a time.
                def wave_slots(n):
                    slots = []
                    i = 0
                    while i + 1 < min(n, 5):
                        pp = psum.tile([128, 2 * LQ], F32, tag="ss", bufs=2,
                                       name="wave_pair")
                        slots += [pp[:, 0:512], pp[:, 512:1024]]
                        i += 2
                    while i < n:
                        ps_w1 = psum.tile([128, LQ], F32, tag="t1", bufs=2,
                                          name="wave_t1")
                        slots.append(ps_w1)
                        i += 1
                    return slots

                def dr_wave(groups, wh, wl, xh, xl, wsl, xsl, finish):
                    slots = wave_slots(len(groups))
                    nt = 0
                    for wt, xt in ((wh, xh), (wh, xl), (wl, xh)):
                        for kp in range(NKP):
                            for ps, g in zip(slots, groups):
                                nc.tensor.matmul(
                                    ps, wsl(wt, kp, g), xsl(xt, kp, g),
                                    start=(nt == 0), stop=(nt == 11),
                                    perf_mode=DR)
                            nt += 1
                    for ps, g in zip(slots, groups):
                        finish(ps, g)

                def q_finish(ps, mc):
                    nc.scalar.activation(qT[:, mc, :], ps, CPY, scale=QSC)

                dr_wave(
                    list(range(6)), wqch, wqcl, xqTh, xqTl,
                    lambda w, kp, mc: w[:, mc, 2 * kp:2 * kp + 2, :],
                    lambda x, kp, mc: x[:, 2 * kp:2 * kp + 2, :],
                    q_finish)
                # xkv lo loads issue on the Act queue mid-q-proj: keeps the
                # HWDGE free for xq/xkv-hi early, lands before k-proj hl terms
                for kc in range(NKC):
                    xload(xkvl, xkl_d, kc, nc.scalar)
                dr_wave(
                    [6, 7], wqch, wqcl, xqTh, xqTl,
                    lambda w, kp, mc: w[:, mc, 2 * kp:2 * kp + 2, :],
                    lambda x, kp, mc: x[:, 2 * kp:2 * kp + 2, :],
                    q_finish)

                # k projection
                def k_finish(ps, g):
                    mc, nh = g
                    nc.vector.tensor_scalar_mul(kT[:, mc, ts(nh, 512)], ps, KSC)

                kgroups = [(mc, nh) for mc in range(NKC) for nh in range(2)]
                for w0 in (0, 6, 12):
                    dr_wave(
                        kgroups[w0:w0 + 6], wmkh, wmkl, xkvh, xkvl,
                        lambda w, kp, g: w[:, g[0], 2 * kp:2 * kp + 2, :],
                        lambda x, kp, g: x[:, 2 * kp:2 * kp + 2, ts(g[1], 512)],
                        k_finish)

                for h in range(4):
                    issue_srow(h)
                nc.sync.dma_start(out=ident, in_=id_d)

                # v projection interleaved with heads 0-7's scores/exp/
                # bias-mul (two scores units per v-group): front-loading 8
                # heads of the Act-bound exp pipeline into phase B keeps Act
                # from pacing phase C. The attn matmuls for these heads run
                # in C (at_queue).
                vgroups = [(jc, nh) for jc in range(NJC) for nh in range(2)]
                vi = 0
                for c in range(4):
                    for hh in range(2):
                        h = 2 * c + hh
                        srow_sb = srow_tiles.pop(h)
                        issue_srow(h + 4)
                        for jp in range(NJC // 2):
                            scores_unit(c, hh, jp, srow_sb)
                            if jp % 2 == 1:
                                jcv, nh = vgroups[vi]
                                vi += 1
                                ps = psum.tile([128, 512], F32, tag="t1", bufs=2)
                                dr_group(
                                    ps, xkvh, xkvl, wmvh, wmvl,
                                    lambda x, kp, jcv=jcv: x[:, 2 * kp:2 * kp + 2, ts(jcv, 128)],
                                    lambda w, kp, nh=nh: w[:, 2 * kp:2 * kp + 2, ts(nh, 512)])
                                dst = vaug[:, jcv, nh * 8:(nh + 1) * 8, 0:64]
                                nc.vector.tensor_scalar_mul(dst, ps, VSC)

            # ================= phase C: attention =================
            with tc.tile_pool(name="phC", bufs=1) as phc:
                # gate-phase weights load during C on the Pool SWDGE queue
                # (it is idle between per-head lo-subtracts), chunked by kc
                # so the transfers spread across DMA engines, and issued one
                # pair per head block so they never form an issue convoy in
                # front of time-critical work.
                wgrh = phc.tile([128, NKC, 2 * D], FP8)
                wgrl = phc.tile([128, NKC, 2 * D], FP8)
                bgt_sb = phc.tile([128, 2 * NKC], F32)
                nc.gpsimd.dma_start(out=bgt_sb, in_=bg_d)

                wg_chunks = [(t, kc) for kc in range(NKC) for t in range(2)]

                def issue_wg(n=2):
                    for _ in range(n):
                        if wg_chunks:
                            t, kc = wg_chunks.pop(0)
                            dst, src = ((wgrh, wgh_d), (wgrl, wgl_d))[t]
                            nc.gpsimd.dma_start(out=dst[:, kc], in_=src[:, kc])

                # Attention runs TRANSPOSED: out[i, hd] with wT as the
                # stationary and vaug [128j, 65] as the moving tensor, so
                # each accumulation matmul costs 65 rows instead of 512
                # (the matmul cost is the moving free size). Bonuses: the
                # softmax denominator lands as a per-partition column, so
                # normalization is a single tensor_scalar with a [128,1]
                # scalar AP (no ones-broadcast matmul, no rb copies), and a
                # paired PE transpose rebuilds the [d, i] layout the gate
                # needs - writing even heads to partitions 0-63 and odd to
                # 64-127 directly (no cross-partition DMA).
                def at_group(c, hh, ic):
                    h = 2 * c + hh
                    ps_at = psum.tile([128, 65], F32, tag="at", bufs=2,
                                      name="ps_at")
                    for jc in range(NJC):
                        nc.tensor.matmul(
                            ps_at, pending[(h, jc)][:, ts(ic, 128)],
                            vaug[:, jc, h, :],
                            start=(jc == 0), stop=(jc == NJC - 1))
                    rsb = phc.tile([128, 1], F32, tag="rsb", bufs=4,
                                   name="rsb")
                    with nc.allow_low_precision(reason="softmax recip"):
                        nc.vector.reciprocal(rsb, ps_at[:, 64:65])
                        nc.vector.tensor_scalar_mul(
                            attnT[:, ic, h, :], ps_at[:, 0:64], rsb)
                    if ic == NIC - 1:
                        for jc in range(NJC):
                            pending.pop((h, jc))

                def finish_pair(c):
                    # transpose both heads' normalized attn back to [d, i]
                    # and split hi/lo fp8 for the gate
                    for ic in range(NIC):
                        pt = psum.tile([128, 128], mdt, tag="t1", bufs=2,
                                       name="pt")
                        nc.tensor.transpose(pt, attnT[:, ic, 2 * c:2 * c + 2, :], ident)
                        with nc.allow_low_precision(reason="gate fp8 hi/lo"):
                            nc.scalar.activation(attn_h[:, c, ts(ic, 128)], pt,
                                                 CPY)
                            nc.vector.tensor_tensor(
                                attn_l[:, c, ts(ic, 128)], pt,
                                attn_h[:, c, ts(ic, 128)], SUB)

                # pending attn-T groups: emitted one per scores_unit so each
                # head's attn matmuls interleave with the NEXT head's
                # scores/exp pipeline
                at_queue = []

                def emit_at(n):
                    for _ in range(n):
                        if not at_queue:
                            return
                        c0, hh0, ic0 = at_queue.pop(0)
                        at_group(c0, hh0, ic0)
                        if hh0 == 1 and ic0 == NIC - 1:
                            finish_pair(c0)

                # the interleaved heads' wT are ready; queue their groups
                for c in range(4):
                    for hh in range(2):
                        at_queue += [(c, hh, ic) for ic in range(NIC)]

                for c in range(4, NKC):
                    for hh in range(2):
                        h = 2 * c + hh
                        srow_sb = srow_tiles.pop(h)
                        if h + 4 < H:
                            issue_srow(h + 4)
                        issue_wg()
                        for jp in range(NJC // 2):
                            scores_unit(c, hh, jp, srow_sb)
                            emit_at(2)
                        at_queue += [(c, hh, ic) for ic in range(NIC)]
                emit_at(len(at_queue))

                # ================= phase D: gate (transposed) =================
                # out[cols, i] = Wg[:, cols].T @ attn: with gate columns on
                # PARTITIONS the bias is per-partition, so the sigmoid fuses
                # scale+bias into the Act instruction, the a-half fuses them
                # into one DVE tensor_scalar, and the host un-transposes the
                # [D, LQ] output for free.
                def gate_drT(ps, colsl):
                    # kp-major, kp3 (heads 12-15, finished last) at the end
                    n = 0
                    for kp in range(NKP):
                        for wt, xt in ((wgrh, attn_h), (wgrh, attn_l), (wgrl, attn_h)):
                            nc.tensor.matmul(
                                ps, wt[:, 2 * kp:2 * kp + 2, colsl],
                                xt[:, 2 * kp:2 * kp + 2, :],
                                start=(n == 0), stop=(n == 11), perf_mode=DR)
                            n += 1

                for cc in range(NKC):
                    bsl = slice(D + cc * 128, D + cc * 128 + 128)
                    ps_b = psum.tile([128, 512], F32, tag="t1", bufs=2)
                    gate_drT(ps_b, bsl)
                    tsg = phc.tile([128, 512], F32, tag="tsg", bufs=2)
                    nc.scalar.activation(tsg, ps_b, SIG, scale=GSC,
                                         bias=bgt_sb[:, 8 + cc:9 + cc])

                    asl = slice(cc * 128, cc * 128 + 128)
                    ps_a = psum.tile([128, 512], F32, tag="t1", bufs=2)
                    gate_drT(ps_a, asl)
                    if cc < NKC - 1:
                        ta = phc.tile([128, 512], F32, tag="ta", bufs=2)
                        nc.vector.tensor_scalar(
                            ta, ps_a, GSC, bgt_sb[:, cc:cc + 1], MUL, ADD)
                        outh = phc.tile([128, 512], mdt, tag="outt", bufs=3)
                        nc.vector.tensor_tensor(outh, ta, tsg, MUL)
                        nc.sync.dma_start(out=out_d[ts(cc, 128), :], in_=outh)
                    else:
                        # final col-chunk in 128-token pieces, DMAs
                        # alternating SP/Act issue, for a short tail
                        for qt in range(4):
                            sl = slice(qt * 128, qt * 128 + 128)
                            ta = phc.tile([128, 128], F32, tag="ta2", bufs=4)
                            nc.vector.tensor_scalar(
                                ta, ps_a[:, sl], GSC, bgt_sb[:, cc:cc + 1],
                                MUL, ADD)
                            outh = phc.tile([128, 128], mdt, tag="outt2", bufs=4)
                            nc.vector.tensor_tensor(outh, ta, tsg[:, sl], MUL)
                            eng = nc.sync if qt % 2 == 0 else nc.scalar
                            eng.dma_start(out=out_d[ts(cc, 128), sl], in_=outh)

    nc.compile()
    return nc


# ======================= host side =======================

def _tisa_ebias(amp, off, sharp):
    d = np.arange(-(L - 1), L, dtype=np.float32)
    s = np.sum(
        amp[:, :, None].astype(np.float32)
        * np.exp(-np.abs(sharp)[:, :, None].astype(np.float32)
                 * (d[None, None, :] - off[:, :, None].astype(np.float32)) ** 2),
        axis=1, dtype=np.float32).astype(np.float32)
    return np.exp(s).astype(np.float32)


def _hilo(x, s):
    """Split x*s into fp8 hi + lo (both e4m3)."""
    xs = (x * s).astype(np.float32)
    hi = xs.astype(NPF8)
    lo = (xs - hi.astype(np.float32)).astype(NPF8)
    return hi, lo


def make_host_inputs(inputs, cfg="fp8"):
    npdt = ml_dtypes.bfloat16
    x_q = np.asarray(inputs["x_q"])
    x_kv = np.asarray(inputs["x_kv"])
    wq = np.asarray(inputs["Wq"])
    wm = np.asarray(inputs["Wm"])
    wg = np.asarray(inputs["Wg"])
    bg = np.asarray(inputs["bg"]).astype(np.float32)

    ebias = _tisa_ebias(np.asarray(inputs["tisa_amp"]),
                        np.asarray(inputs["tisa_off"]),
                        np.asarray(inputs["tisa_sharp"]))

    p_i = np.arange(128)[:, None]
    m_i = np.arange(SROW_W)[None, :]
    srows = []
    for i_off in (0, 512):
        idx = p_i - m_i + (1919 - i_off)
        srows.append(np.ascontiguousarray(ebias[:, idx]).astype(npdt))

    def colchunk(w):
        # [k, m] -> [mc, p, kc*128]: [mc][p][kc*128+c] = w[kc*128+p, mc*128+c]
        return np.ascontiguousarray(
            w.reshape(NKC, 128, NKC, 128).transpose(2, 1, 0, 3).reshape(NKC, 128, D))

    def prow(w):
        # [k, cols] -> [p, kc, cols]
        return np.ascontiguousarray(
            w.reshape(NKC, 128, w.shape[1]).transpose(1, 0, 2))

    wqh, wql = _hilo(wq, SW)
    wmkh, wmkl = _hilo(wm[:, :D], SW)
    wmvh, wmvl = _hilo(wm[:, D:], SW)
    wgh, wgl = _hilo(wg, SW)

    bgt = np.ascontiguousarray(bg.reshape(2 * NKC, 128).T).astype(np.float32)

    in_maps = []
    for core in range(8):
        b, half = core // 2, core % 2
        xq = np.ascontiguousarray(x_q[b, half * LQ:(half + 1) * LQ].T)
        xkv = np.ascontiguousarray(x_kv[b].T)
        xqh, xql = _hilo(xq, SX)
        xkh, xkl = _hilo(xkv, SX)
        in_maps.append({
            "xqh": xqh, "xql": xql,
            "xkh": xkh, "xkl": xkl,
            "wqh": colchunk(wqh), "wql": colchunk(wql),
            "wmkh": colchunk(wmkh), "wmkl": colchunk(wmkl),
            "wmvh": prow(wmvh), "wmvl": prow(wmvl),
            "wgh": prow(wgh), "wgl": prow(wgl),
            "srow": srows[half],
            "bgt": bgt,
            "ident": np.eye(128, dtype=npdt),
        })
    return in_maps


def assemble_output(results):
    out = np.empty((4, L, D), dtype=np.float32)
    for core in range(8):
        b, half = core // 2, core % 2
        out[b, half * LQ:(half + 1) * LQ] = results[core]["out"].astype(np.float32).T
    return out


# ======================= public entry point =======================

_NC_CACHE = {}


def _get_nc(cfg):
    if cfg not in _NC_CACHE:
        _NC_CACHE[cfg] = build_nc(cfg)
    return _NC_CACHE[cfg]


def kernel(**inputs):
    """Full (unsharded) inputs -> full (4, 1024, 1024) float32 output.

    Shards over 8 NeuronCores: core = (batch, query-half). Host precomputes
    the TISA exp-bias lookup table, pre-transposes activations, and splits
    activations/weights into compensated fp8 hi/lo pairs; projections and
    gate run as fp8 DoubleRow matmuls, attention in bf16, all with fp32
    accumulation.
    """
    from concourse.bass_utils import run_bass_kernel_spmd

    cfg = "fp8"
    nc = _get_nc(cfg)
    in_maps = make_host_inputs(inputs, cfg)
    res = run_bass_kernel_spmd(nc, in_maps, core_ids=list(range(8)))
    return assemble_output(res.results)
